# revision 1
# baseline (speedup 1.0000x reference)
"""Trainium2 Bass kernel for nn_DiffKS (differentiable Karplus-Strong).

Structure of the computation:
  y[t] = x[t] - sum_{j=0..5} vals[t,j] * y[t - 1 - z_l[t] - j]
with vals / z_l derived from spline-interpolated delay & coefficient
trajectories.  The feedback lag (1 + z_l + j) is always >= ~93 samples, so
128-sample chunks can be computed as dense banded matmuls against a
512-sample window of past output (4 ring columns of 128, partition-aligned)
plus a small within-chunk correction term.

8-core strategy (the recurrence is strictly sequential in time; there is a
single voice, so we parallelise over time segments using linearity):
  - split the 65536 samples into 8 segments of 8192 (one per NeuronCore)
  - phase B (parallel): every core runs its segment's chunked recurrence
    with basis+1 right-hand sides (basis = max feedback lag, ~427): unit
    "basis" initial-window columns + 1 particular column (the excitation
    with zero initial window).  This yields, per chunk, its response
    operator H_c (stored to DRAM), and per segment its transfer operator:
    final-window = T[:, :basis] @ initial-window + T[:, basis].
  - combine (host, tiny): chain the 8 transfer operators to get every
    segment's true initial window w_s (8 small matvecs).
  - apply (parallel): y[:, c] = H_c @ [w_s; 1] — one fused multiply+
    accumulate-reduce per chunk on the Vector engine, plus within-chunk
    correction fix-ups.

Performance notes: all-zero 128x128 weight blocks are skipped (shared SPMD
plans = per-position union across the 8 segments); each chunk's
within-chunk correction is algebraically folded into the weights of its
downstream readers on the host (_fold_corr), which removes the correction
matmul from the serial chunk-to-chunk dependency chain; DIFFKS_FASTB=1
optionally runs phase B matmuls in float32r (FP22 multiplies, ~1.25x
faster end-to-end, relative error 2.7e-4 instead of 1.6e-5).
"""

import os
import numpy as np

import concourse.bacc as bacc
import concourse.tile as tile
import concourse.mybir as mybir
from concourse.bass_utils import run_bass_kernel_spmd


def _ensure_ntff_hook():
    """The agent image's `antenv` stub lacks `axon_hooks`, which
    `run_bass_kernel_spmd(trace=True)` needs under axon for NTFF capture.
    Recreate the same ctypes-based hook `trn_agent_boot.trn_boot` would
    install on images where the module exists (see its section 6)."""
    try:
        from antenv.axon_hooks import get_axon_ntff_profile_hook  # noqa: F401
        return
    except ImportError:
        pass
    import contextlib
    import ctypes
    import sys
    import types

    so_path = "/opt/axon/libaxon_pjrt.so"
    if not os.path.exists(so_path):
        return
    lib = ctypes.CDLL(so_path)
    if not hasattr(lib, "axon_start_nrt_profile"):
        return
    lib.axon_start_nrt_profile.argtypes = [
        ctypes.POINTER(ctypes.c_int64), ctypes.c_size_t]
    lib.axon_start_nrt_profile.restype = ctypes.c_int64
    lib.axon_stop_nrt_profile.argtypes = [ctypes.c_char_p]
    lib.axon_stop_nrt_profile.restype = ctypes.c_int64

    @contextlib.contextmanager
    def _hook(output_dir, device_ids):
        import jax
        jax.devices()
        if device_ids:
            ids = (ctypes.c_int64 * len(device_ids))(*device_ids)
            rc = lib.axon_start_nrt_profile(ids, len(device_ids))
        else:
            rc = lib.axon_start_nrt_profile(None, 0)
        if rc != 0:
            raise RuntimeError(f"axon_start_nrt_profile rc={rc}")
        try:
            yield
        finally:
            n = lib.axon_stop_nrt_profile(str(output_dir).encode())
            if n <= 0:
                print(f"ntff profile: {n} file(s) written to {output_dir}",
                      file=sys.stderr)

    mod = types.ModuleType("antenv.axon_hooks")
    mod._hook = _hook
    mod.get_axon_ntff_profile_hook = lambda: _hook
    mod.set_axon_ntff_profile_hook = lambda h: setattr(mod, "_hook", h)
    import antenv
    antenv.axon_hooks = mod
    sys.modules["antenv.axon_hooks"] = mod


_ensure_ntff_hook()

F32 = mybir.dt.float32

N_SAMPLES = 65536
N_FRAMES = 64
L_ORDER = 5
CHUNK = 128
WIN = 512            # window length the chunk matmuls see (4 ring cols)
RING = 8             # ring columns in SBUF (power of two, >= 5)
CORR = 64            # within-chunk correction width (needs z_l >= 63)
BASIS = 448          # basis = window positions 64..511 (needs z_l <= 442)
NRHS = BASIS + 1
N_CORES = 8

# filled by kernel() with per-phase profiling results for the test harness
LAST_RESULTS = {}

# built bass programs, keyed by structure — repeated kernel() calls with the
# same inputs reuse the same program objects (and their compile caches)
_NC_CACHE = {}


# ----------------------------------------------------------------------------
# host-side preprocessing (input-independent spline matrix + tiny elementwise)
# ----------------------------------------------------------------------------

_SPLINE_CACHE = {}


def _spline_matrix(n_in, n_out):
    """Static [n_out, n_in] natural-cubic-spline interpolation matrix for
    uniform knots t_in=linspace(0,1,n_in) evaluated at linspace(0,1,n_out).
    Input-independent (depends only on the fixed shapes)."""
    key = (n_in, n_out)
    if key in _SPLINE_CACHE:
        return _SPLINE_CACHE[key]
    t_in = np.linspace(0.0, 1.0, n_in)
    t_out = np.linspace(0.0, 1.0, n_out)
    n = n_in
    h = t_in[1:] - t_in[:-1]
    R = np.zeros((n - 2, n))
    for i in range(n - 2):
        R[i, i] += 6.0 / h[i]
        R[i, i + 1] += -6.0 / h[i] - 6.0 / h[i + 1]
        R[i, i + 2] += 6.0 / h[i + 1]
    A = (
        np.diag(2.0 * (h[:-1] + h[1:]))
        + np.diag(h[1:-1], 1)
        + np.diag(h[1:-1], -1)
    )
    M = np.zeros((n, n))
    M[1:-1] = np.linalg.solve(A, R)          # second derivatives, linear in y
    idx = np.clip(np.searchsorted(t_in, t_out, side="right") - 1, 0, n - 2)
    dt = t_out - t_in[idx]
    S = np.zeros((n_out, n))
    eye = np.eye(n)
    for r in range(n_out):
        i = idx[r]
        b = (eye[i + 1] - eye[i]) / h[i] - h[i] * (2.0 * M[i] + M[i + 1]) / 6.0
        c = M[i] / 2.0
        d = (M[i + 1] - M[i]) / (6.0 * h[i])
        S[r] = eye[i] + b * dt[r] + c * dt[r] ** 2 + d * dt[r] ** 3
    S = S.astype(np.float32)
    _SPLINE_CACHE[key] = S
    return S


def _preprocess(delay, raw, exc, n_samples):
    sig = 1.0 / (1.0 + np.exp(-np.asarray(raw, np.float32)))
    coeff = sig / sig.sum(-1, keepdims=True)
    S = _spline_matrix(N_FRAMES, n_samples)
    delay_interp = S @ np.asarray(delay, np.float32)
    coeff_interp = S @ coeff
    z_l = np.floor(delay_interp).astype(np.int32)
    alfa = (delay_interp - z_l).astype(np.float32)
    b = coeff_interp
    v0 = -(1.0 - alfa) * b[:, 0]
    vmid = -(alfa[:, None] * b[:, : L_ORDER - 1]
             + (1.0 - alfa)[:, None] * b[:, 1:L_ORDER])
    vL = -alfa * b[:, -1]
    vals = np.concatenate([v0[:, None], vmid, vL[:, None]], 1).astype(np.float32)
    x = np.zeros(n_samples, np.float32)
    exc = np.asarray(exc, np.float32)
    x[: exc.shape[0]] = exc
    return vals, z_l, x


def _build_wts(vals, z_l, n_samples):
    """Dense per-chunk matmul weights, already transposed into lhsT layout.

    Returns (wts, basis) with wts [n_chunks, 5*128, 128] fp32 where:
      wts[c, 128g + p, m] = W[c][m, 128g + p]   (g = 0..3, window blocks)
      wts[c, 512 + p, m]  = L[c][m, p]          (p < 64, correction block)
    W[c][i, k] multiplies window sample y[128c - 512 + k] into output i;
    L[c][i, k] multiplies within-chunk y[128c + k] (k < 64) into output i.
    basis = max feedback lag (the needed width of the window basis)."""
    n_chunks = n_samples // CHUNK
    t = np.arange(n_samples)
    lag = 1 + z_l[:, None] + np.arange(6)[None, :]       # [T, 6]
    assert (lag[:, 0] >= CORR).all(), "delay too small for correction width"
    basis = int(lag.max())
    assert basis <= WIN - CORR, "delay too large for window"
    src = t[:, None] - lag                                # absolute read pos
    i_in_chunk = t % CHUNK
    k_win = WIN + i_in_chunk[:, None] - lag               # window col if < WIN
    wts = np.zeros((n_chunks, 5 * CHUNK, CHUNK), np.float32)
    c_of_t = t // CHUNK
    for j in range(6):
        valid = src[:, j] >= 0
        kw = k_win[:, j]
        in_window = valid & (kw < WIN)
        # window part: wts[c, kw, i] = vals[t, j]
        tw = t[in_window]
        wts[c_of_t[tw], kw[tw], i_in_chunk[tw]] += vals[tw, j]
        in_chunk = valid & (kw >= WIN)
        tc = t[in_chunk]
        kc = kw[tc] - WIN
        assert (kc < CORR).all()
        wts[c_of_t[tc], WIN + kc, i_in_chunk[tc]] += vals[tc, j]
    return wts, basis


def _fold_corr(wts_seg):
    """Fold each chunk's within-chunk correction into the weights of its
    in-segment readers, so the ring can store *uncorrected* columns and the
    correction matmul leaves the serial chunk-to-chunk dependency chain.

    Stored column of a corr-active chunk w: rows < CORR are true, rows >=
    CORR carry +psum2 = +(Lc @ y_lo).  A reader's true contribution is
    W @ true = W @ stored - W[:, CORR:] @ Lc[CORR:, :CORR] @ stored[:CORR],
    so fold:  lhsT_B[0:CORR] -= LcT[0:CORR, CORR:] @ lhsT_B[CORR:].
    Exact algebra; modifies wts_seg in place and returns it."""
    wts_seg = wts_seg.copy()
    n = wts_seg.shape[0]
    blocks = wts_seg.reshape(n, 5, CHUNK, CHUNK)
    corr_active = np.abs(blocks[:, 4]).reshape(n, -1).max(-1) > 0
    for w in range(n):
        if not corr_active[w]:
            continue
        corrT = blocks[w, 4]                      # [p, m] = Lc[m, p]
        for r in range(w + 1, min(w + 5, n)):
            g = w - r + 4
            blk = blocks[r, g]
            blk[0:CORR] -= corrT[0:CORR, CORR:] @ blk[CORR:]
    return wts_seg


def _union_plans(wts_segs, corr_pos_fn):
    """Shared (SPMD) per-position plans = union of active blocks across the
    per-core segments, plus per-phase correction positions.

    wts_segs: list of per-core [cps, 640, 128] arrays (already folded).
    corr_pos_fn(pos, corr_union) -> bool: whether position `pos` carries a
    correction matmul in the program.
    Returns (plans, packed_list): plans[c] = (wblocks, corr);
    packed_list[s] = [n_blocks, 128, 128] for core s (zero-padded where that
    core's block is inactive)."""
    cps = wts_segs[0].shape[0]
    act = np.stack([
        np.abs(w.reshape(cps, 5, -1)).max(-1) > 0 for w in wts_segs
    ])                                            # [n_seg, cps, 5]
    union = act.any(0)                            # [cps, 5]
    plans = []
    for c in range(cps):
        wblocks = [g for g in range(4) if union[c, g]]
        if not wblocks:
            wblocks = [3]
        plans.append((wblocks, bool(corr_pos_fn(c, union[c, 4]))))
    packed_list = []
    for w in wts_segs:
        blocks = w.reshape(cps, 5, CHUNK, CHUNK)
        out = []
        for c, (wblocks, corr) in enumerate(plans):
            sel = list(wblocks) + ([4] if corr else [])
            out.append(blocks[c, sel])
        packed_list.append(np.ascontiguousarray(np.concatenate(out, 0)))
    return plans, packed_list


# ----------------------------------------------------------------------------
# bass program builder
# ----------------------------------------------------------------------------

def _build_recur_nc(plans, n_blocks, nrhs, basis, want_y, want_t,
                    fast_mm=False, want_h=False):
    key = ("recur", tuple((tuple(wb), co) for wb, co in plans), n_blocks,
           nrhs, basis, want_y, want_t, fast_mm, want_h)
    if key in _NC_CACHE:
        return _NC_CACHE[key]
    nc = _build_recur_nc_impl(plans, n_blocks, nrhs, basis, want_y, want_t,
                              fast_mm, want_h)
    _NC_CACHE[key] = nc
    return nc


def _build_recur_nc_impl(plans, n_blocks, nrhs, basis, want_y, want_t,
                         fast_mm, want_h):
    """Bass/Tile program running the chunked recurrence with `nrhs`
    right-hand-side columns; per-chunk blocks given by `plans`.

    The ring stores *uncorrected* columns (corrections are folded into the
    reader weights on the host — see _fold_corr), so the chunk-to-chunk
    serial chain is just matmul -> subtract.  Where plans[c] includes the
    correction block, a correction matmul computes the true values off the
    chain for the outputs (yout / final window).

    Inputs:  wts   [n_blocks, 128, 128] f32  (packed lhsT blocks)
             xin   [128, n_chunks]      f32  (x, chunk-column layout)
             ring0 [128, 4, nrhs]       f32  (initial window columns, true)
    Outputs: tout  [128, 4, nrhs]       f32  (true final window, if want_t)
             yout  [128, n_chunks]      f32  (true outputs, if want_y)
    """
    n_chunks = len(plans)
    nb_max = max(len(wb) + int(co) for wb, co in plans)
    # float32r: PE reads fp32 bits but multiplies at FP22 in a single pass
    # (vs 2 half-speed passes for true fp32) — ~4x faster at wide N.
    MMDT = mybir.dt.float32r if fast_mm else F32
    nc = bacc.Bacc("TRN2", target_bir_lowering=False, debug=False,
                   num_devices=N_CORES, enable_partition_id=False)
    wts = nc.dram_tensor("wts", [n_blocks, CHUNK, CHUNK], MMDT,
                         kind="ExternalInput")
    xin = nc.dram_tensor("xin", [CHUNK, n_chunks], F32, kind="ExternalInput")
    ring0 = nc.dram_tensor("ring0", [CHUNK, 4, nrhs], MMDT,
                           kind="ExternalInput")
    tout = yout = hout = None
    if want_t:
        tout = nc.dram_tensor("tout", [CHUNK, 4, nrhs], F32,
                              kind="ExternalOutput")
    if want_y:
        yout = nc.dram_tensor("yout", [CHUNK, n_chunks], F32,
                              kind="ExternalOutput")
    if want_h:
        # uncorrected response operators (= ring columns), 4 chunks a batch
        hout = nc.dram_tensor("hout", [n_chunks // 4, CHUNK, 4, nrhs], MMDT,
                              kind="ExternalOutput")

    with tile.TileContext(nc) as tc:
        with (
            tc.tile_pool(name="state", bufs=1) as state,
            tc.tile_pool(name="wpool", bufs=8) as wpool,
            tc.tile_pool(name="psum", bufs=4, space="PSUM") as ppool,
        ):
            ring = state.tile([CHUNK, RING, nrhs], MMDT)
            xin_sb = state.tile([CHUNK, n_chunks], F32)
            # ring0 first: the first chunk's matmuls need it, xin can wait
            nc.sync.dma_start(ring[:, 4:8, :], ring0[:])
            nc.sync.dma_start(xin_sb[:], xin[:])
            yout_sb = trueout = xext = None
            if want_y:
                yout_sb = state.tile([CHUNK, n_chunks], F32)
            if want_t:
                trueout = state.tile([CHUNK, 4, nrhs], F32)
            if nrhs > 1:
                # x-extended rhs template: zeros except the particular col
                xext = state.tile([CHUNK, nrhs], F32)
                nc.vector.memset(xext[:], 0.0)

            lo = slice(0, CORR)
            hi = slice(CORR, CHUNK)
            off = 0
            for c in range(n_chunks):
                wblocks, corr = plans[c]
                nb = len(wblocks) + int(corr)
                wtile = wpool.tile([CHUNK, nb_max, CHUNK], MMDT, tag="wt")
                nc.sync.dma_start(
                    wtile[:, 0:nb, :],
                    wts[off: off + nb].rearrange("b p m -> p b m"),
                )
                off += nb
                psum = ppool.tile([CHUNK, nrhs], F32, tag="acc")
                for i, g in enumerate(wblocks):
                    col = (c + 4 + g) % RING
                    nc.tensor.matmul(
                        psum[:],
                        wtile[:, i, :],
                        ring[:, col, :],
                        start=(i == 0),
                        stop=(i == len(wblocks) - 1),
                    )
                rc = c % RING
                # stored (uncorrected) column — the serial chain tail
                if nrhs == 1:
                    nc.vector.tensor_sub(
                        ring[:, rc, :], xin_sb[:, c: c + 1], psum[:]
                    )
                else:
                    # refresh the particular column of the template (off the
                    # chain), then one fused op: ring_col = -psum + xext
                    nc.vector.tensor_copy(
                        xext[:, basis: basis + 1], xin_sb[:, c: c + 1])
                    nc.vector.scalar_tensor_tensor(
                        out=ring[:, rc, :], in0=psum[:], scalar=-1.0,
                        in1=xext[:], op0=mybir.AluOpType.mult,
                        op1=mybir.AluOpType.add,
                    )
                if want_h and c % 4 == 3:
                    base = (c - 3) % RING          # 0 or 4: contiguous 4 cols
                    nc.sync.dma_start(
                        hout[c // 4], ring[:, base: base + 4, :])
                # corrected outputs, off the chain
                psum2 = None
                if corr:
                    psum2 = ppool.tile([CHUNK, nrhs], F32, tag="corr")
                    nc.tensor.matmul(
                        psum2[:],
                        wtile[0:CORR, nb - 1, :],
                        ring[lo, rc, :],
                        start=True,
                        stop=True,
                    )
                if want_y:
                    if corr:
                        nc.vector.tensor_copy(
                            yout_sb[lo, c: c + 1], ring[lo, rc, :])
                        nc.vector.tensor_sub(
                            yout_sb[hi, c: c + 1], ring[hi, rc, :],
                            psum2[hi, :])
                    else:
                        nc.vector.tensor_copy(
                            yout_sb[:, c: c + 1], ring[:, rc, :])
                if want_t and c >= n_chunks - 4:
                    k = c - (n_chunks - 4)
                    if corr:
                        nc.vector.tensor_copy(
                            trueout[lo, k, :], ring[lo, rc, :])
                        nc.vector.tensor_sub(
                            trueout[hi, k, :], ring[hi, rc, :], psum2[hi, :])
                    else:
                        nc.vector.tensor_copy(trueout[:, k, :], ring[:, rc, :])

            assert n_chunks % RING == 0
            if want_t:
                nc.sync.dma_start(tout[:], trueout[:])
            if want_y:
                nc.sync.dma_start(yout[:], yout_sb[:])
    nc.compile()
    return nc


def _build_apply_nc(corr_flags, nrhs, fast_h=False):
    key = ("apply", tuple(corr_flags), nrhs)
    if key in _NC_CACHE:
        return _NC_CACHE[key]
    nc = _build_apply_nc_impl(corr_flags, nrhs)
    _NC_CACHE[key] = nc
    return nc


def _build_apply_nc_impl(corr_flags, nrhs):
    """Bass/Tile program applying the segment's true initial window to the
    stored per-chunk response operators from phase B:
        y[:, c] = H_c @ wvec      (fused multiply+reduce on the vector engine)
    then fixing the within-chunk correction for corr-active chunks.

    Inputs:  hseg  [n_chunks, 128, nrhs]  (phase B's hout)
             wb    [128, nrhs]            (wvec broadcast across partitions)
             cwts  [n_corr, 128, 128]     (correction lhsT blocks, packed)
    Outputs: yout  [128, n_chunks]
    """
    n_chunks = len(corr_flags)
    n_corr = int(np.sum(corr_flags))
    assert n_chunks % 4 == 0
    # H bytes are fp32 either way (float32r is just an fp32 PE read mode)
    HDT = F32
    nc = bacc.Bacc("TRN2", target_bir_lowering=False, debug=False,
                   num_devices=N_CORES, enable_partition_id=False)
    hseg = nc.dram_tensor("hseg", [n_chunks // 4, CHUNK, 4, nrhs], HDT,
                          kind="ExternalInput")
    wb = nc.dram_tensor("wb", [CHUNK, nrhs], F32, kind="ExternalInput")
    cwts = nc.dram_tensor("cwts", [max(n_corr, 1), CHUNK, CHUNK], F32,
                          kind="ExternalInput")
    yout = nc.dram_tensor("yout", [CHUNK, n_chunks], F32,
                          kind="ExternalOutput")

    with tile.TileContext(nc) as tc:
        with (
            tc.tile_pool(name="state", bufs=1) as state,
            tc.tile_pool(name="hpool", bufs=8) as hpool,
            tc.tile_pool(name="spool", bufs=4) as spool,
            tc.tile_pool(name="psum", bufs=4, space="PSUM") as ppool,
        ):
            wb_sb = state.tile([CHUNK, nrhs], F32)
            nc.sync.dma_start(wb_sb[:], wb[:])
            # all correction blocks stay resident in SBUF
            call_sb = state.tile([CHUNK, max(n_corr, 1), CHUNK], F32)
            nc.sync.dma_start(
                call_sb[:], cwts[:].rearrange("b p m -> p b m"))
            yout_sb = state.tile([CHUNK, n_chunks], F32)
            # pass 1: all the multiply+reduce work, back-to-back on DVE
            # (keeping the correction fix-ups out of the in-order DVE stream
            # here avoids head-of-line blocking behind the PE matmuls)
            for c0 in range(0, n_chunks, 4):
                htile = hpool.tile([CHUNK, 4, nrhs], HDT, tag="h")
                nc.sync.dma_start(htile[:], hseg[c0 // 4])
                for c in range(c0, c0 + 4):
                    # the elementwise product is discarded (only the fp32
                    # accumulator is used); bf16 out may enable a faster
                    # DVE mode without touching accumulation precision
                    scratch = spool.tile([CHUNK, nrhs], mybir.dt.bfloat16,
                                         tag="s")
                    nc.vector.scalar_tensor_tensor(
                        out=scratch[:], in0=htile[:, c - c0, :], scalar=1.0,
                        in1=wb_sb[:], op0=mybir.AluOpType.mult,
                        op1=mybir.AluOpType.mult,
                        accum_out=yout_sb[:, c: c + 1],
                    )
            # pass 2: correction fix-ups
            ci = 0
            for c in range(n_chunks):
                if not corr_flags[c]:
                    continue
                psum2 = ppool.tile([CHUNK, 1], F32, tag="corr")
                nc.tensor.matmul(
                    psum2[:], call_sb[0:CORR, ci, :],
                    yout_sb[0:CORR, c: c + 1],
                    start=True, stop=True,
                )
                ci += 1
                nc.vector.tensor_sub(
                    yout_sb[CORR:CHUNK, c: c + 1],
                    yout_sb[CORR:CHUNK, c: c + 1],
                    psum2[CORR:CHUNK, :],
                )
            assert ci == n_corr
            nc.sync.dma_start(yout[:], yout_sb[:])
    nc.compile()
    return nc


# ----------------------------------------------------------------------------
# host orchestration
# ----------------------------------------------------------------------------

def _run(nc, in_maps, tag):
    trace = bool(int(os.environ.get("DIFFKS_TRACE", "0")))
    kw = {}
    tcs = os.environ.get("DIFFKS_TRACE_CORES", "")
    if trace and tcs:
        kw["trace_cores"] = [int(x) for x in tcs.split(",")]
    res = run_bass_kernel_spmd(
        nc, in_maps, core_ids=list(range(len(in_maps))), trace=trace, **kw
    )
    LAST_RESULTS[tag] = res
    return res.results


def _basis_ring0(basis):
    """Initial window columns for phase B: basis b is a unit vector at
    window position (WIN-basis)+b; the particular column starts at zero."""
    nrhs = basis + 1
    r0 = np.zeros((CHUNK, 4, nrhs), np.float32)
    for b in range(basis):
        p = (WIN - basis) + b
        r0[p % CHUNK, p // CHUNK, b] = 1.0
    return r0


def kernel(delay_len_frames, raw_coeff_frames, excitation, n_samples):
    n = int(n_samples)
    assert n == N_SAMPLES, f"kernel hardcoded for {N_SAMPLES}, got {n}"
    LAST_RESULTS.clear()

    vals, z_l, x = _preprocess(delay_len_frames, raw_coeff_frames,
                               excitation, n)
    wts, basis = _build_wts(vals, z_l, n)
    nrhs = basis + 1
    n_chunks = n // CHUNK
    xin = np.ascontiguousarray(x.reshape(n_chunks, CHUNK).T)   # [128, n_chunks]

    mode = os.environ.get("DIFFKS_MODE", "seg")
    if mode == "seq":
        # single-chain: every core runs the full sequence; take core 0
        folded = _fold_corr(wts)
        plans, packed = _union_plans([folded], lambda c, cu: cu)
        nc = _build_recur_nc(plans, packed[0].shape[0], 1, basis,
                             want_y=True, want_t=False)
        in_map = {
            "wts": packed[0],
            "xin": xin,
            "ring0": np.zeros((CHUNK, 4, 1), np.float32),
        }
        outs = _run(nc, [in_map] * N_CORES, "seq")
        y = outs[0]["yout"].T.reshape(n)                        # [128, nc] -> t
        return y.astype(np.float32)

    # ---- segmented: 8 cores, phase B -> host combine -> phase C ----
    cps = n_chunks // N_CORES                                   # chunks/segment
    seg_wts = [_fold_corr(wts[s * cps: (s + 1) * cps])
               for s in range(N_CORES)]
    seg_xin = [np.ascontiguousarray(xin[:, s * cps: (s + 1) * cps])
               for s in range(N_CORES)]

    # phase B: basis + particular responses; correction only needed for the
    # final window columns (last 4 chunk positions)
    fast_b = bool(int(os.environ.get("DIFFKS_FASTB", "0")))
    use_apply = bool(int(os.environ.get("DIFFKS_APPLY", "1")))
    plansB, packedB = _union_plans(
        seg_wts, lambda c, cu: cu and c >= cps - 4)
    ncB = _build_recur_nc(plansB, packedB[0].shape[0], nrhs, basis,
                          want_y=False, want_t=True, fast_mm=fast_b,
                          want_h=use_apply)
    r0 = _basis_ring0(basis)
    in_maps = [
        {"wts": packedB[s], "xin": seg_xin[s], "ring0": r0}
        for s in range(N_CORES)
    ]
    outsB = _run(ncB, in_maps, "phaseB")

    # host combine: chain transfer operators (8 tiny matvecs)
    wins = [np.zeros(WIN, np.float32)]
    for s in range(N_CORES):
        T = outsB[s]["tout"]            # [128, 4, nrhs]
        T = T.transpose(1, 0, 2).reshape(WIN, nrhs)   # window pos major
        w_next = T[:, :basis] @ wins[s][WIN - basis:] + T[:, basis]
        wins.append(w_next.astype(np.float32))

    if use_apply:
        # apply pass: y[:, c] = H_c @ [w; 1], plus correction fix-ups
        corr_flags = [
            bool(np.any([
                np.abs(w.reshape(cps, 5, CHUNK, CHUNK)[c, 4]).max() > 0
                for w in seg_wts
            ]))
            for c in range(cps)
        ]
        ncA = _build_apply_nc(corr_flags, nrhs, fast_h=fast_b)
        n_corr = max(int(np.sum(corr_flags)), 1)
        in_maps = []
        for s in range(N_CORES):
            blocks = seg_wts[s].reshape(cps, 5, CHUNK, CHUNK)
            cw = np.zeros((n_corr, CHUNK, CHUNK), np.float32)
            ci = 0
            for c in range(cps):
                if corr_flags[c]:
                    cw[ci] = blocks[c, 4]
                    ci += 1
            wv = np.concatenate(
                [wins[s][WIN - basis:], np.ones(1, np.float32)])
            wb = np.ascontiguousarray(
                np.broadcast_to(wv, (CHUNK, nrhs))).astype(np.float32)
            in_maps.append({
                "hseg": outsB[s]["hout"],
                "wb": wb,
                "cwts": cw,
            })
        outsC = _run(ncA, in_maps, "apply")
    else:
        # phase C: re-run with the true initial windows
        plansC, packedC = _union_plans(seg_wts, lambda c, cu: cu)
        ncC = _build_recur_nc(plansC, packedC[0].shape[0], 1, basis,
                              want_y=True, want_t=False)
        in_maps = [
            {
                "wts": packedC[s],
                "xin": seg_xin[s],
                "ring0": np.ascontiguousarray(
                    wins[s].reshape(4, CHUNK).T.reshape(CHUNK, 4, 1)
                ),
            }
            for s in range(N_CORES)
        ]
        outsC = _run(ncC, in_maps, "phaseC")

    y = np.concatenate(
        [outsC[s]["yout"].T.reshape(cps * CHUNK) for s in range(N_CORES)]
    )
    return y.astype(np.float32)



# revision 16
# speedup vs baseline: 4.3180x; 4.3180x over previous
"""Trainium2 Bass kernel for nn_DiffKS (differentiable Karplus-Strong).

Structure of the computation:
  y[t] = x[t] - sum_{j=0..5} vals[t,j] * y[t - 1 - z_l[t] - j]
with vals / z_l derived from spline-interpolated delay & coefficient
trajectories.  The feedback lag (1 + z_l + j) is always >= ~93 samples, so
128-sample chunks can be computed as dense banded matmuls against a
512-sample window of past output plus a small within-chunk correction.

Parallel structure (v2):
  - 32 time segments of 2048 samples; each of the 8 cores runs 4 segments
    as INDEPENDENT interleaved chunk-chains (4 chains x 16 rounds), which
    keeps the tensor engine busy while each chain's serial
    matmul->ring-update link completes.  All tensors fp16 (weights, ring,
    H) with fp32 PSUM accumulation: host-simulated rel err ~5e-4.
  - phase B (parallel): each chain runs its segment's chunked recurrence
    with basis+1 right-hand sides (unit initial-window columns + one
    particular column).  Ring columns double as the response operator H
    (streamed to DRAM in fp16); the corrected final windows form the
    segment's transfer operator T.
  - combine (host, tiny): chain the 32 transfer operators to get every
    segment's true initial window.
  - apply (parallel): y[:, c] = H_c @ [w; 1] as fused multiply+
    accumulate-reduce ops split across the Vector and GpSimd engines,
    plus within-chunk correction fix-ups.

Weights are pre-negated on the host so the serial ring update is a plain
PSUM->SBUF copy on the Scalar (ACT) engine (plus a [128,1] x-column add on
Vector), keeping the Vector engine off the critical chain.
"""

import os
import numpy as np

import concourse.bacc as bacc
import concourse.tile as tile
import concourse.mybir as mybir
from concourse.bass_utils import run_bass_kernel_spmd


def _ensure_ntff_hook():
    """The agent image's `antenv` stub lacks `axon_hooks`, which
    `run_bass_kernel_spmd(trace=True)` needs under axon for NTFF capture."""
    try:
        from antenv.axon_hooks import get_axon_ntff_profile_hook  # noqa: F401
        return
    except ImportError:
        pass
    import contextlib
    import ctypes
    import sys
    import types

    so_path = "/opt/axon/libaxon_pjrt.so"
    if not os.path.exists(so_path):
        return
    lib = ctypes.CDLL(so_path)
    if not hasattr(lib, "axon_start_nrt_profile"):
        return
    lib.axon_start_nrt_profile.argtypes = [
        ctypes.POINTER(ctypes.c_int64), ctypes.c_size_t]
    lib.axon_start_nrt_profile.restype = ctypes.c_int64
    lib.axon_stop_nrt_profile.argtypes = [ctypes.c_char_p]
    lib.axon_stop_nrt_profile.restype = ctypes.c_int64

    @contextlib.contextmanager
    def _hook(output_dir, device_ids):
        import jax
        jax.devices()
        if device_ids:
            ids = (ctypes.c_int64 * len(device_ids))(*device_ids)
            rc = lib.axon_start_nrt_profile(ids, len(device_ids))
        else:
            rc = lib.axon_start_nrt_profile(None, 0)
        if rc != 0:
            raise RuntimeError(f"axon_start_nrt_profile rc={rc}")
        try:
            yield
        finally:
            n = lib.axon_stop_nrt_profile(str(output_dir).encode())
            if n <= 0:
                print(f"ntff profile: {n} file(s) written to {output_dir}",
                      file=sys.stderr)

    mod = types.ModuleType("antenv.axon_hooks")
    mod._hook = _hook
    mod.get_axon_ntff_profile_hook = lambda: _hook
    mod.set_axon_ntff_profile_hook = lambda h: setattr(mod, "_hook", h)
    import antenv
    antenv.axon_hooks = mod
    sys.modules["antenv.axon_hooks"] = mod


_ensure_ntff_hook()

F32 = mybir.dt.float32
F16 = mybir.dt.float16

N_SAMPLES = 65536
N_FRAMES = 64
L_ORDER = 5
CHUNK = 128
WIN = 512            # window length the chunk matmuls see (4 ring cols)
RING = 8             # ring columns per chain in SBUF
CORR = 64            # within-chunk correction width (needs z_l >= 63)
N_CORES = 8
CH = 4               # independent chains (segments) per core
CPC = 16             # chunks (rounds) per chain
SEGS = N_CORES * CH  # 32 segments of 2048 samples

# filled by kernel() with per-phase profiling results for the test harness
LAST_RESULTS = {}

_NC_CACHE = {}

# device rhs layout: basis columns [0, basis); one zero pad column; the
# particular column at PIDX (4-byte aligned for the fp16 column update);
# one trailing pad so the total width is even.
def _nr_of(basis):
    pidx = basis + 1 + (basis + 1) % 2
    return pidx + 2 - (basis + 1) % 2, pidx


def _seg_of(s, q):
    """Segment index handled by core s, chain q."""
    return s + N_CORES * q


# ----------------------------------------------------------------------------
# host-side preprocessing
# ----------------------------------------------------------------------------

_SPLINE_CACHE = {}


def _spline_matrix(n_in, n_out):
    """Static [n_out, n_in] natural-cubic-spline interpolation matrix."""
    key = (n_in, n_out)
    if key in _SPLINE_CACHE:
        return _SPLINE_CACHE[key]
    t_in = np.linspace(0.0, 1.0, n_in)
    t_out = np.linspace(0.0, 1.0, n_out)
    n = n_in
    h = t_in[1:] - t_in[:-1]
    R = np.zeros((n - 2, n))
    for i in range(n - 2):
        R[i, i] += 6.0 / h[i]
        R[i, i + 1] += -6.0 / h[i] - 6.0 / h[i + 1]
        R[i, i + 2] += 6.0 / h[i + 1]
    A = (
        np.diag(2.0 * (h[:-1] + h[1:]))
        + np.diag(h[1:-1], 1)
        + np.diag(h[1:-1], -1)
    )
    M = np.zeros((n, n))
    M[1:-1] = np.linalg.solve(A, R)
    idx = np.clip(np.searchsorted(t_in, t_out, side="right") - 1, 0, n - 2)
    dt = t_out - t_in[idx]
    S = np.zeros((n_out, n))
    eye = np.eye(n)
    for r in range(n_out):
        i = idx[r]
        b = (eye[i + 1] - eye[i]) / h[i] - h[i] * (2.0 * M[i] + M[i + 1]) / 6.0
        c = M[i] / 2.0
        d = (M[i + 1] - M[i]) / (6.0 * h[i])
        S[r] = eye[i] + b * dt[r] + c * dt[r] ** 2 + d * dt[r] ** 3
    S = S.astype(np.float32)
    _SPLINE_CACHE[key] = S
    return S


def _preprocess(delay, raw, exc, n_samples):
    sig = 1.0 / (1.0 + np.exp(-np.asarray(raw, np.float32)))
    coeff = sig / sig.sum(-1, keepdims=True)
    S = _spline_matrix(N_FRAMES, n_samples)
    delay_interp = S @ np.asarray(delay, np.float32)
    coeff_interp = S @ coeff
    z_l = np.floor(delay_interp).astype(np.int32)
    alfa = (delay_interp - z_l).astype(np.float32)
    b = coeff_interp
    v0 = -(1.0 - alfa) * b[:, 0]
    vmid = -(alfa[:, None] * b[:, : L_ORDER - 1]
             + (1.0 - alfa)[:, None] * b[:, 1:L_ORDER])
    vL = -alfa * b[:, -1]
    vals = np.concatenate([v0[:, None], vmid, vL[:, None]], 1).astype(np.float32)
    x = np.zeros(n_samples, np.float32)
    exc = np.asarray(exc, np.float32)
    x[: exc.shape[0]] = exc
    return vals, z_l, x


def _build_wts(vals, z_l, n_samples):
    """Dense per-chunk matmul weights in lhsT layout.

    wts[c, 128g + p, m] = W[c][m, 128g + p]   (g = 0..3, window blocks)
    wts[c, 512 + p, m]  = L[c][m, p]          (p < 64, correction block)
    """
    n_chunks = n_samples // CHUNK
    t = np.arange(n_samples)
    lag = 1 + z_l[:, None] + np.arange(6)[None, :]
    assert (lag[:, 0] >= CORR).all(), "delay too small for correction width"
    basis = int(lag.max())
    assert basis <= WIN - CORR, "delay too large for window"
    src = t[:, None] - lag
    i_in_chunk = t % CHUNK
    k_win = WIN + i_in_chunk[:, None] - lag
    wts = np.zeros((n_chunks, 5 * CHUNK, CHUNK), np.float32)
    c_of_t = t // CHUNK
    for j in range(6):
        valid = src[:, j] >= 0
        kw = k_win[:, j]
        in_window = valid & (kw < WIN)
        tw = t[in_window]
        wts[c_of_t[tw], kw[tw], i_in_chunk[tw]] += vals[tw, j]
        in_chunk = valid & (kw >= WIN)
        tc = t[in_chunk]
        kc = kw[tc] - WIN
        assert (kc < CORR).all()
        wts[c_of_t[tc], WIN + kc, i_in_chunk[tc]] += vals[tc, j]
    return wts, basis


def _fold_corr(wts_seg):
    """Fold each chunk's within-chunk correction into the weights of its
    in-segment readers so the ring stores *uncorrected* columns."""
    wts_seg = wts_seg.copy()
    n = wts_seg.shape[0]
    blocks = wts_seg.reshape(n, 5, CHUNK, CHUNK)
    corr_active = np.abs(blocks[:, 4]).reshape(n, -1).max(-1) > 0
    for w in range(n):
        if not corr_active[w]:
            continue
        corrT = blocks[w, 4]
        for r in range(w + 1, min(w + 5, n)):
            g = w - r + 4
            blk = blocks[r, g]
            blk[0:CORR] -= corrT[0:CORR, CORR:] @ blk[CORR:]
    return wts_seg


def _basis_ring0(basis, nr):
    """Initial window columns: basis b is a unit at window position
    (WIN-basis)+b; particular and pad columns start at zero."""
    r0 = np.zeros((CHUNK, 4, nr), np.float32)
    for b in range(basis):
        p = (WIN - basis) + b
        r0[p % CHUNK, p // CHUNK, b] = 1.0
    return r0


# ----------------------------------------------------------------------------
# plan construction (shared across cores; SPMD program)
# ----------------------------------------------------------------------------

def _make_plans(seg_wts_neg):
    """Per-slot (q, r) union plans across cores.

    Returns:
      plans[q][r] = (wblocks tuple, corr_t bool)   # phase B
      corr_y[q][r] = bool                          # apply fixup positions
    """
    act = np.stack([
        np.abs(w.reshape(CPC, 5, -1)).max(-1) > 0 for w in seg_wts_neg
    ])  # [SEGS, CPC, 5]
    plans = []
    corr_y = []
    for q in range(CH):
        segs = [_seg_of(s, q) for s in range(N_CORES)]
        u = act[segs].any(0)  # [CPC, 5]
        pq = []
        cq = []
        for r in range(CPC):
            wb = tuple(g for g in range(4) if u[r, g])
            if not wb:
                wb = (3,)
            pq.append((wb, False))
            cq.append(bool(u[r, 4]))
        plans.append(pq)
        corr_y.append(cq)
    return plans, corr_y


def _pack_weights(seg_wts_neg, plans, s):
    """Pack core s's phase-B weight blocks, round-major, partition-major
    fp16 layout [128, TOT, 128]."""
    cols = []
    for r in range(CPC):
        for q in range(CH):
            wb, corr_t = plans[q][r]
            blocks = seg_wts_neg[_seg_of(s, q)].reshape(CPC, 5, CHUNK, CHUNK)
            sel = list(wb) + ([4] if corr_t else [])
            cols.append(blocks[r, sel])  # [nb, 128, 128]
    flat = np.concatenate(cols, 0)       # [TOT, 128k, 128m]
    return np.ascontiguousarray(
        flat.transpose(1, 0, 2)).astype(np.float16)  # [128, TOT, 128]


# ----------------------------------------------------------------------------
# phase B program
# ----------------------------------------------------------------------------

def _plan_key(plans):
    return tuple(tuple((wb, co) for wb, co in pq) for pq in plans)


def _build_phaseb_nc(plans, tot_blocks, basis):
    upd = os.environ.get("DIFFKS_UPD", "act")   # act | stt | vv
    hq = os.environ.get("DIFFKS_HQ", "gpsimd")  # gpsimd | sync
    usplit_env = os.environ.get("DIFFKS_USPLIT", "344")
    key = ("B2", _plan_key(plans), tot_blocks, basis, upd, hq, usplit_env)
    if key in _NC_CACHE:
        return _NC_CACHE[key]
    nr, pidx = _nr_of(basis)

    nb_round_max = max(
        sum(len(plans[q][r][0]) + int(plans[q][r][1]) for q in range(CH))
        for r in range(CPC)
    )
    nc = bacc.Bacc("TRN2", target_bir_lowering=False, debug=False,
                   num_devices=N_CORES, enable_partition_id=False)
    wts = nc.dram_tensor("wts", [CHUNK, tot_blocks, CHUNK], F16,
                         kind="ExternalInput")
    xin = nc.dram_tensor("xin", [CHUNK, CH * CPC], F32, kind="ExternalInput")
    ring0 = nc.dram_tensor("ring0", [CHUNK, 4, nr], F16,
                           kind="ExternalInput")
    hout = nc.dram_tensor("hout", [CHUNK, CH * CPC, nr], F16,
                          kind="ExternalOutput")

    with tile.TileContext(nc) as tc:
        with (
            tc.tile_pool(name="state", bufs=1) as state,
            tc.tile_pool(name="wpool", bufs=4) as wpool,
            tc.tile_pool(name="psum", bufs=8, space="PSUM") as ppool,
        ):
            ring = state.tile([CHUNK, CH, RING, nr], F16)
            xin_sb = state.tile([CHUNK, CH * CPC], F32)
            xext = None
            if upd in ("stt", "vv"):
                xext = state.tile([CHUNK, CH, nr], F16)
                nc.vector.memset(xext[:], 0.0)
            for q in range(CH):
                nc.sync.dma_start(ring[:, q, 4:8, :], ring0[:])
                if upd == "act":
                    # written columns never touch the trailing pad column;
                    # zero it so H stays NaN-free for the apply pass
                    nc.vector.memset(ring[:, q, 0:4, pidx + 1:], 0.0)
            nc.sync.dma_start(xin_sb[:], xin[:])

            woff = 0
            for r in range(CPC):
                nbr = sum(len(plans[q][r][0]) + int(plans[q][r][1])
                          for q in range(CH))
                wtile = wpool.tile([CHUNK, nb_round_max, CHUNK], F16, tag="w")
                nc.sync.dma_start(wtile[:, 0:nbr, :],
                                  wts[:, woff: woff + nbr, :])
                woff += nbr
                soff = 0
                for q in range(CH):
                    wb, corr_t = plans[q][r]
                    slot = r * CH + q
                    rc = r % RING
                    psum = ppool.tile([CHUNK, nr], F32, tag="acc")
                    for i, g in enumerate(wb):
                        col = (r + 4 + g) % RING
                        nc.tensor.matmul(
                            psum[:],
                            wtile[:, soff + i, :],
                            ring[:, q, col, :],
                            start=(i == 0),
                            stop=(i == len(wb) - 1),
                        )
                    # serial ring update (weights pre-negated: col = psum + x)
                    if upd == "act":
                        usplit = int(os.environ.get("DIFFKS_USPLIT", "344"))
                        nc.scalar.copy(ring[:, q, rc, 0:usplit],
                                       psum[:, 0:usplit])
                        if usplit < pidx:
                            nc.vector.tensor_copy(
                                ring[:, q, rc, usplit:pidx],
                                psum[:, usplit:pidx])
                        nc.vector.tensor_add(
                            ring[:, q, rc, pidx: pidx + 1],
                            psum[:, pidx: pidx + 1],
                            xin_sb[:, slot: slot + 1],
                        )
                    else:
                        if upd == "stt":
                            nc.scalar.copy(xext[:, q, pidx: pidx + 1],
                                           xin_sb[:, slot: slot + 1])
                        else:
                            nc.vector.tensor_copy(xext[:, q, pidx: pidx + 1],
                                                  xin_sb[:, slot: slot + 1])
                        nc.vector.scalar_tensor_tensor(
                            out=ring[:, q, rc, :], in0=psum[:], scalar=1.0,
                            in1=xext[:, q, :], op0=mybir.AluOpType.mult,
                            op1=mybir.AluOpType.add,
                        )
                    soff += len(wb) + int(corr_t)
                    if r % 4 == 3:
                        base = (r - 3) % RING
                        dmaeng = nc.gpsimd if hq == "gpsimd" else nc.sync
                        dmaeng.dma_start(
                            hout[:, q * CPC + (r - 3): q * CPC + r + 1, :],
                            ring[:, q, base: base + 4, :],
                        )
    nc.compile()
    _NC_CACHE[key] = nc
    return nc


# ----------------------------------------------------------------------------
# apply program
# ----------------------------------------------------------------------------

def _build_apply_nc(corr_slots, nrhs):
    acc = os.environ.get("DIFFKS_ACC", "vec")   # split | vec
    ncorrmode = os.environ.get("DIFFKS_NCORR", "on")  # on | off
    key = ("A2", tuple(corr_slots), nrhs, acc, ncorrmode)
    if key in _NC_CACHE:
        return _NC_CACHE[key]
    n_corr = max(len(corr_slots), 1)
    nc = bacc.Bacc("TRN2", target_bir_lowering=False, debug=False,
                   num_devices=N_CORES, enable_partition_id=False)
    hseg = nc.dram_tensor("hseg", [CHUNK, CH * CPC, nrhs], F16,
                          kind="ExternalInput")
    wb = nc.dram_tensor("wb", [CHUNK, CH, nrhs], F16, kind="ExternalInput")
    cwts = nc.dram_tensor("cwts", [CORR, n_corr, CHUNK], F16,
                          kind="ExternalInput")
    yout = nc.dram_tensor("yout", [CHUNK, CH * CPC], F32,
                          kind="ExternalOutput")

    with tile.TileContext(nc) as tc:
        with (
            tc.tile_pool(name="state", bufs=1) as state,
            tc.tile_pool(name="hpool", bufs=6) as hpool,
            tc.tile_pool(name="spool", bufs=4) as spool,
            tc.tile_pool(name="lpool", bufs=4) as lpool,
            tc.tile_pool(name="psum", bufs=6, space="PSUM") as ppool,
        ):
            wb_sb = state.tile([CHUNK, CH, nrhs], F16)
            nc.sync.dma_start(wb_sb[:], wb[:])
            cw_sb = state.tile([CORR, n_corr, CHUNK], F16)
            nc.sync.dma_start(cw_sb[:], cwts[:])
            yout_sb = state.tile([CHUNK, CH * CPC], F32)
            # fused multiply + accumulate-reduce per H column, with each
            # corr fix-up issued right after its column (PE/ACT work overlaps
            # the DVE accumulation stream)
            ci_of = {hcol: ci for ci, hcol in enumerate(corr_slots)}
            for grp in range(CH * CPC // 4):
                htile = hpool.tile([CHUNK, 4, nrhs], F16, tag="h")
                nc.sync.dma_start(htile[:],
                                  hseg[:, grp * 4:(grp + 1) * 4, :])
                for j in range(4):
                    hcol = grp * 4 + j
                    q = hcol // CPC
                    scratch = spool.tile([CHUNK, nrhs], F16, tag="s")
                    if acc == "ttr":
                        nc.vector.tensor_tensor_reduce(
                            out=scratch[:], in0=htile[:, j, :],
                            in1=wb_sb[:, q, :], scale=1.0, scalar=0.0,
                            op0=mybir.AluOpType.mult,
                            op1=mybir.AluOpType.add,
                            accum_out=yout_sb[:, hcol: hcol + 1],
                        )
                    else:
                        nc.vector.scalar_tensor_tensor(
                            out=scratch[:], in0=htile[:, j, :], scalar=1.0,
                            in1=wb_sb[:, q, :], op0=mybir.AluOpType.mult,
                            op1=mybir.AluOpType.mult,
                            accum_out=yout_sb[:, hcol: hcol + 1],
                        )
                    if ncorrmode == "on" and hcol in ci_of:
                        ci = ci_of[hcol]
                        ylo = lpool.tile([CORR, 1], F16, tag="lo")
                        nc.scalar.copy(ylo[:], yout_sb[0:CORR, hcol: hcol + 1])
                        psum2 = ppool.tile([CHUNK, 1], F32, tag="c")
                        nc.tensor.matmul(psum2[:], cw_sb[:, ci, :], ylo[:],
                                         start=True, stop=True)
                        nc.vector.tensor_add(
                            yout_sb[CORR:, hcol: hcol + 1],
                            yout_sb[CORR:, hcol: hcol + 1],
                            psum2[CORR:, :],
                        )
            nc.sync.dma_start(yout[:], yout_sb[:])
    nc.compile()
    _NC_CACHE[key] = nc
    return nc


# ----------------------------------------------------------------------------
# host orchestration
# ----------------------------------------------------------------------------

def _run(nc, in_maps, tag):
    trace = bool(int(os.environ.get("DIFFKS_TRACE", "0")))
    kw = {}
    tcs = os.environ.get("DIFFKS_TRACE_CORES", "")
    if trace and tcs:
        kw["trace_cores"] = [int(x) for x in tcs.split(",")]
    res = run_bass_kernel_spmd(
        nc, in_maps, core_ids=list(range(len(in_maps))), trace=trace, **kw
    )
    LAST_RESULTS[tag] = res
    return res.results


def kernel(delay_len_frames, raw_coeff_frames, excitation, n_samples):
    n = int(n_samples)
    assert n == N_SAMPLES, f"kernel hardcoded for {N_SAMPLES}, got {n}"
    LAST_RESULTS.clear()

    vals, z_l, x = _preprocess(delay_len_frames, raw_coeff_frames,
                               excitation, n)
    wts, basis = _build_wts(vals, z_l, n)
    nr, pidx = _nr_of(basis)
    n_chunks = n // CHUNK
    assert n_chunks == SEGS * CPC
    xin_cols = np.ascontiguousarray(x.reshape(n_chunks, CHUNK).T)  # [128, nc]

    # fold corrections, then negate everything (update becomes plain copy)
    seg_wts_neg = [-_fold_corr(wts[j * CPC:(j + 1) * CPC])
                   for j in range(SEGS)]
    plans, corr_y = _make_plans(seg_wts_neg)
    tot_blocks = sum(len(plans[q][r][0]) + int(plans[q][r][1])
                     for r in range(CPC) for q in range(CH))

    ncB = _build_phaseb_nc(plans, tot_blocks, basis)
    r0 = _basis_ring0(basis, nr).astype(np.float16)
    in_maps = []
    for s in range(N_CORES):
        xin = np.zeros((CHUNK, CH * CPC), np.float32)
        for r in range(CPC):
            for q in range(CH):
                gchunk = _seg_of(s, q) * CPC + r
                xin[:, r * CH + q] = xin_cols[:, gchunk]
        in_maps.append({
            "wts": _pack_weights(seg_wts_neg, plans, s),
            "xin": xin,
            "ring0": r0,
        })
    outsB = _run(ncB, in_maps, "phaseB")

    # host combine: build each segment's transfer operator from its last 4
    # (uncorrected) H columns + the correction blocks, then chain them (fp32)
    wins = [np.zeros(WIN, np.float32)]
    for j in range(SEGS):
        s, q = j % N_CORES, j // N_CORES
        base = q * CPC + (CPC - 4)
        T = outsB[s]["hout"][:, base: base + 4, :].astype(np.float32)
        blocks = seg_wts_neg[j].reshape(CPC, 5, CHUNK, CHUNK)
        for k in range(4):
            Lc = blocks[CPC - 4 + k, 4][0:CORR]      # negated lhsT [64, 128]
            if np.any(Lc):
                fix = Lc.T @ T[0:CORR, k, :]          # [128, nr]
                T[CORR:, k, :] += fix[CORR:]
        T = T.transpose(1, 0, 2).reshape(WIN, nr)
        w_next = T[:, :basis] @ wins[j][WIN - basis:] + T[:, pidx]
        wins.append(w_next.astype(np.float32))

    # apply: y[:, c] = H_c @ [w; 1]
    corr_slots = [q * CPC + r for q in range(CH) for r in range(CPC)
                  if corr_y[q][r]]
    ncA = _build_apply_nc(corr_slots, nr)
    n_corr = max(len(corr_slots), 1)
    in_maps = []
    for s in range(N_CORES):
        wbv = np.zeros((CHUNK, CH, nr), np.float16)
        for q in range(CH):
            j = _seg_of(s, q)
            wv = np.zeros(nr, np.float32)
            wv[:basis] = wins[j][WIN - basis:]
            wv[pidx] = 1.0
            wbv[:, q, :] = wv.astype(np.float16)[None, :]
        cw = np.zeros((CORR, n_corr, CHUNK), np.float16)
        for ci, hcol in enumerate(corr_slots):
            q, r = hcol // CPC, hcol % CPC
            blocks = seg_wts_neg[_seg_of(s, q)].reshape(CPC, 5, CHUNK, CHUNK)
            cw[:, ci, :] = blocks[r, 4][0:CORR].astype(np.float16)
        in_maps.append({
            "hseg": outsB[s]["hout"],
            "wb": wbv,
            "cwts": cw,
        })
    outsA = _run(ncA, in_maps, "apply")

    y = np.zeros(n, np.float32)
    for s in range(N_CORES):
        yo = outsA[s]["yout"]          # [128, CH*CPC]
        for q in range(CH):
            for r in range(CPC):
                gchunk = _seg_of(s, q) * CPC + r
                y[gchunk * CHUNK:(gchunk + 1) * CHUNK] = yo[:, q * CPC + r]
    return y.astype(np.float32)


# revision 20
# speedup vs baseline: 4.3573x; 1.0091x over previous
"""Trainium2 Bass kernel for nn_DiffKS (differentiable Karplus-Strong).

Structure of the computation:
  y[t] = x[t] - sum_{j=0..5} vals[t,j] * y[t - 1 - z_l[t] - j]
with vals / z_l derived from spline-interpolated delay & coefficient
trajectories.  The feedback lag (1 + z_l + j) is always >= ~93 samples, so
128-sample chunks can be computed as dense banded matmuls against a
512-sample window of past output plus a small within-chunk correction.

Parallel structure (v2):
  - 32 time segments of 2048 samples; each of the 8 cores runs 4 segments
    as INDEPENDENT interleaved chunk-chains (4 chains x 16 rounds), which
    keeps the tensor engine busy while each chain's serial
    matmul->ring-update link completes.  All tensors fp16 (weights, ring,
    H) with fp32 PSUM accumulation: host-simulated rel err ~5e-4.
  - phase B (parallel): each chain runs its segment's chunked recurrence
    with basis+1 right-hand sides (unit initial-window columns + one
    particular column).  Ring columns double as the response operator H
    (streamed to DRAM in fp16); the corrected final windows form the
    segment's transfer operator T.
  - combine (host, tiny): chain the 32 transfer operators to get every
    segment's true initial window.
  - apply (parallel): y[:, c] = H_c @ [w; 1] as fused multiply+
    accumulate-reduce ops split across the Vector and GpSimd engines,
    plus within-chunk correction fix-ups.

Weights are pre-negated on the host so the serial ring update is a plain
PSUM->SBUF copy on the Scalar (ACT) engine (plus a [128,1] x-column add on
Vector), keeping the Vector engine off the critical chain.
"""

import os
import numpy as np

import concourse.bacc as bacc
import concourse.tile as tile
import concourse.mybir as mybir
from concourse.bass_utils import run_bass_kernel_spmd


def _ensure_ntff_hook():
    """The agent image's `antenv` stub lacks `axon_hooks`, which
    `run_bass_kernel_spmd(trace=True)` needs under axon for NTFF capture."""
    try:
        from antenv.axon_hooks import get_axon_ntff_profile_hook  # noqa: F401
        return
    except ImportError:
        pass
    import contextlib
    import ctypes
    import sys
    import types

    so_path = "/opt/axon/libaxon_pjrt.so"
    if not os.path.exists(so_path):
        return
    lib = ctypes.CDLL(so_path)
    if not hasattr(lib, "axon_start_nrt_profile"):
        return
    lib.axon_start_nrt_profile.argtypes = [
        ctypes.POINTER(ctypes.c_int64), ctypes.c_size_t]
    lib.axon_start_nrt_profile.restype = ctypes.c_int64
    lib.axon_stop_nrt_profile.argtypes = [ctypes.c_char_p]
    lib.axon_stop_nrt_profile.restype = ctypes.c_int64

    @contextlib.contextmanager
    def _hook(output_dir, device_ids):
        import jax
        jax.devices()
        if device_ids:
            ids = (ctypes.c_int64 * len(device_ids))(*device_ids)
            rc = lib.axon_start_nrt_profile(ids, len(device_ids))
        else:
            rc = lib.axon_start_nrt_profile(None, 0)
        if rc != 0:
            raise RuntimeError(f"axon_start_nrt_profile rc={rc}")
        try:
            yield
        finally:
            n = lib.axon_stop_nrt_profile(str(output_dir).encode())
            if n <= 0:
                print(f"ntff profile: {n} file(s) written to {output_dir}",
                      file=sys.stderr)

    mod = types.ModuleType("antenv.axon_hooks")
    mod._hook = _hook
    mod.get_axon_ntff_profile_hook = lambda: _hook
    mod.set_axon_ntff_profile_hook = lambda h: setattr(mod, "_hook", h)
    import antenv
    antenv.axon_hooks = mod
    sys.modules["antenv.axon_hooks"] = mod


_ensure_ntff_hook()

F32 = mybir.dt.float32
F16 = mybir.dt.float16

N_SAMPLES = 65536
N_FRAMES = 64
L_ORDER = 5
CHUNK = 128
WIN = 512            # window length the chunk matmuls see (4 ring cols)
RING = 8             # ring columns per chain in SBUF
CORR = 64            # within-chunk correction width (needs z_l >= 63)
N_CORES = 8
CH = 4               # independent chains (segments) per core
CPC = 16             # chunks (rounds) per chain
SEGS = N_CORES * CH  # 32 segments of 2048 samples

# filled by kernel() with per-phase profiling results for the test harness
LAST_RESULTS = {}

_NC_CACHE = {}

# device rhs layout: basis columns [0, basis); one zero pad column; the
# particular column at PIDX (4-byte aligned for the fp16 column update);
# one trailing pad so the total width is even.
def _nr_of(basis):
    pidx = basis + 1 + (basis + 1) % 2
    return pidx + 2 - (basis + 1) % 2, pidx


def _seg_of(s, q):
    """Segment index handled by core s, chain q."""
    return s + N_CORES * q


# ----------------------------------------------------------------------------
# host-side preprocessing
# ----------------------------------------------------------------------------

_SPLINE_CACHE = {}


def _spline_matrix(n_in, n_out):
    """Static [n_out, n_in] natural-cubic-spline interpolation matrix."""
    key = (n_in, n_out)
    if key in _SPLINE_CACHE:
        return _SPLINE_CACHE[key]
    t_in = np.linspace(0.0, 1.0, n_in)
    t_out = np.linspace(0.0, 1.0, n_out)
    n = n_in
    h = t_in[1:] - t_in[:-1]
    R = np.zeros((n - 2, n))
    for i in range(n - 2):
        R[i, i] += 6.0 / h[i]
        R[i, i + 1] += -6.0 / h[i] - 6.0 / h[i + 1]
        R[i, i + 2] += 6.0 / h[i + 1]
    A = (
        np.diag(2.0 * (h[:-1] + h[1:]))
        + np.diag(h[1:-1], 1)
        + np.diag(h[1:-1], -1)
    )
    M = np.zeros((n, n))
    M[1:-1] = np.linalg.solve(A, R)
    idx = np.clip(np.searchsorted(t_in, t_out, side="right") - 1, 0, n - 2)
    dt = t_out - t_in[idx]
    S = np.zeros((n_out, n))
    eye = np.eye(n)
    for r in range(n_out):
        i = idx[r]
        b = (eye[i + 1] - eye[i]) / h[i] - h[i] * (2.0 * M[i] + M[i + 1]) / 6.0
        c = M[i] / 2.0
        d = (M[i + 1] - M[i]) / (6.0 * h[i])
        S[r] = eye[i] + b * dt[r] + c * dt[r] ** 2 + d * dt[r] ** 3
    S = S.astype(np.float32)
    _SPLINE_CACHE[key] = S
    return S


def _preprocess(delay, raw, exc, n_samples):
    sig = 1.0 / (1.0 + np.exp(-np.asarray(raw, np.float32)))
    coeff = sig / sig.sum(-1, keepdims=True)
    S = _spline_matrix(N_FRAMES, n_samples)
    delay_interp = S @ np.asarray(delay, np.float32)
    coeff_interp = S @ coeff
    z_l = np.floor(delay_interp).astype(np.int32)
    alfa = (delay_interp - z_l).astype(np.float32)
    b = coeff_interp
    v0 = -(1.0 - alfa) * b[:, 0]
    vmid = -(alfa[:, None] * b[:, : L_ORDER - 1]
             + (1.0 - alfa)[:, None] * b[:, 1:L_ORDER])
    vL = -alfa * b[:, -1]
    vals = np.concatenate([v0[:, None], vmid, vL[:, None]], 1).astype(np.float32)
    x = np.zeros(n_samples, np.float32)
    exc = np.asarray(exc, np.float32)
    x[: exc.shape[0]] = exc
    return vals, z_l, x


def _build_wts(vals, z_l, n_samples):
    """Dense per-chunk matmul weights in lhsT layout.

    wts[c, 128g + p, m] = W[c][m, 128g + p]   (g = 0..3, window blocks)
    wts[c, 512 + p, m]  = L[c][m, p]          (p < 64, correction block)
    """
    n_chunks = n_samples // CHUNK
    t = np.arange(n_samples)
    lag = 1 + z_l[:, None] + np.arange(6)[None, :]
    assert (lag[:, 0] >= CORR).all(), "delay too small for correction width"
    basis = int(lag.max())
    assert basis <= WIN - CORR, "delay too large for window"
    src = t[:, None] - lag
    i_in_chunk = t % CHUNK
    k_win = WIN + i_in_chunk[:, None] - lag
    wts = np.zeros((n_chunks, 5 * CHUNK, CHUNK), np.float32)
    c_of_t = t // CHUNK
    for j in range(6):
        valid = src[:, j] >= 0
        kw = k_win[:, j]
        in_window = valid & (kw < WIN)
        tw = t[in_window]
        wts[c_of_t[tw], kw[tw], i_in_chunk[tw]] += vals[tw, j]
        in_chunk = valid & (kw >= WIN)
        tc = t[in_chunk]
        kc = kw[tc] - WIN
        assert (kc < CORR).all()
        wts[c_of_t[tc], WIN + kc, i_in_chunk[tc]] += vals[tc, j]
    return wts, basis


def _fold_corr(wts_seg):
    """Fold each chunk's within-chunk correction into the weights of its
    in-segment readers so the ring stores *uncorrected* columns."""
    wts_seg = wts_seg.copy()
    n = wts_seg.shape[0]
    blocks = wts_seg.reshape(n, 5, CHUNK, CHUNK)
    corr_active = np.abs(blocks[:, 4]).reshape(n, -1).max(-1) > 0
    for w in range(n):
        if not corr_active[w]:
            continue
        corrT = blocks[w, 4]
        for r in range(w + 1, min(w + 5, n)):
            g = w - r + 4
            blk = blocks[r, g]
            blk[0:CORR] -= corrT[0:CORR, CORR:] @ blk[CORR:]
    return wts_seg


def _basis_ring0(basis, nr):
    """Initial window columns: basis b is a unit at window position
    (WIN-basis)+b; particular and pad columns start at zero."""
    r0 = np.zeros((CHUNK, 4, nr), np.float32)
    for b in range(basis):
        p = (WIN - basis) + b
        r0[p % CHUNK, p // CHUNK, b] = 1.0
    return r0


# ----------------------------------------------------------------------------
# plan construction (shared across cores; SPMD program)
# ----------------------------------------------------------------------------

def _make_plans(seg_wts_neg):
    """Per-slot (q, r) union plans across cores.

    Returns:
      plans[q][r] = (wblocks tuple, corr_t bool)   # phase B
      corr_y[q][r] = bool                          # apply fixup positions
    """
    act = np.stack([
        np.abs(w.reshape(CPC, 5, -1)).max(-1) > 0 for w in seg_wts_neg
    ])  # [SEGS, CPC, 5]
    plans = []
    corr_y = []
    for q in range(CH):
        segs = [_seg_of(s, q) for s in range(N_CORES)]
        u = act[segs].any(0)  # [CPC, 5]
        pq = []
        cq = []
        for r in range(CPC):
            wb = tuple(g for g in range(4) if u[r, g])
            if not wb:
                wb = (3,)
            pq.append((wb, False))
            cq.append(bool(u[r, 4]))
        plans.append(pq)
        corr_y.append(cq)
    return plans, corr_y


def _pack_weights(seg_wts_neg, plans, s):
    """Pack core s's phase-B weight blocks, round-major, partition-major
    fp16 layout [128, TOT, 128]."""
    cols = []
    for r in range(CPC):
        for q in range(CH):
            wb, corr_t = plans[q][r]
            blocks = seg_wts_neg[_seg_of(s, q)].reshape(CPC, 5, CHUNK, CHUNK)
            sel = list(wb) + ([4] if corr_t else [])
            cols.append(blocks[r, sel])  # [nb, 128, 128]
    flat = np.concatenate(cols, 0)       # [TOT, 128k, 128m]
    return np.ascontiguousarray(
        flat.transpose(1, 0, 2)).astype(np.float16)  # [128, TOT, 128]


# ----------------------------------------------------------------------------
# phase B program
# ----------------------------------------------------------------------------

def _plan_key(plans):
    return tuple(tuple((wb, co) for wb, co in pq) for pq in plans)


def _build_phaseb_nc(plans, tot_blocks, basis):
    upd = os.environ.get("DIFFKS_UPD", "act")   # act | stt | vv
    hq = os.environ.get("DIFFKS_HQ", "sync")  # gpsimd | sync
    us_env = os.environ.get("DIFFKS_US1", "216")
    key = ("B2", _plan_key(plans), tot_blocks, basis, upd, hq, us_env)
    if key in _NC_CACHE:
        return _NC_CACHE[key]
    nr, pidx = _nr_of(basis)

    nb_round_max = max(
        sum(len(plans[q][r][0]) + int(plans[q][r][1]) for q in range(CH))
        for r in range(CPC)
    )
    nc = bacc.Bacc("TRN2", target_bir_lowering=False, debug=False,
                   num_devices=N_CORES, enable_partition_id=False)
    wts = nc.dram_tensor("wts", [CHUNK, tot_blocks, CHUNK], F16,
                         kind="ExternalInput")
    xin = nc.dram_tensor("xin", [CHUNK, CH * CPC], F32, kind="ExternalInput")
    ring0 = nc.dram_tensor("ring0", [CHUNK, 4, nr], F16,
                           kind="ExternalInput")
    hout = nc.dram_tensor("hout", [CHUNK, CH * CPC, nr], F16,
                          kind="ExternalOutput")

    with tile.TileContext(nc) as tc:
        with (
            tc.tile_pool(name="state", bufs=1) as state,
            tc.tile_pool(name="wpool", bufs=4) as wpool,
            tc.tile_pool(name="psum", bufs=8, space="PSUM") as ppool,
        ):
            ring = state.tile([CHUNK, CH, RING, nr], F16)
            xin_sb = state.tile([CHUNK, CH * CPC], F32)
            xext = None
            if upd in ("stt", "vv"):
                xext = state.tile([CHUNK, CH, nr], F16)
                nc.vector.memset(xext[:], 0.0)
            for q in range(CH):
                nc.sync.dma_start(ring[:, q, 4:8, :], ring0[:])
            nc.sync.dma_start(xin_sb[:], xin[:])

            woff = 0
            for r in range(CPC):
                nbr = sum(len(plans[q][r][0]) + int(plans[q][r][1])
                          for q in range(CH))
                wtile = wpool.tile([CHUNK, nb_round_max, CHUNK], F16, tag="w")
                nc.sync.dma_start(wtile[:, 0:nbr, :],
                                  wts[:, woff: woff + nbr, :])
                woff += nbr
                soff = 0
                for q in range(CH):
                    wb, corr_t = plans[q][r]
                    slot = r * CH + q
                    rc = r % RING
                    psum = ppool.tile([CHUNK, nr], F32, tag="acc")
                    for i, g in enumerate(wb):
                        col = (r + 4 + g) % RING
                        nc.tensor.matmul(
                            psum[:],
                            wtile[:, soff + i, :],
                            ring[:, q, col, :],
                            start=(i == 0),
                            stop=(i == len(wb) - 1),
                        )
                    # serial ring update (weights pre-negated: col = psum + x).
                    # One balanced copy each on ACT and Vector; x is nonzero
                    # only for the first 4 chunks of segment 0, so only those
                    # slots get an in-place x-add (other cores add zero).
                    if upd == "act":
                        s1 = int(os.environ.get("DIFFKS_US1", "216"))
                        nc.scalar.copy(ring[:, q, rc, 0:s1], psum[:, 0:s1])
                        nc.vector.tensor_copy(ring[:, q, rc, s1:nr],
                                              psum[:, s1:nr])
                        if q == 0 and r < 4:
                            nc.vector.tensor_add(
                                ring[:, q, rc, pidx: pidx + 1],
                                ring[:, q, rc, pidx: pidx + 1],
                                xin_sb[:, slot: slot + 1],
                            )
                    else:
                        if upd == "stt":
                            nc.scalar.copy(xext[:, q, pidx: pidx + 1],
                                           xin_sb[:, slot: slot + 1])
                        else:
                            nc.vector.tensor_copy(xext[:, q, pidx: pidx + 1],
                                                  xin_sb[:, slot: slot + 1])
                        nc.vector.scalar_tensor_tensor(
                            out=ring[:, q, rc, :], in0=psum[:], scalar=1.0,
                            in1=xext[:, q, :], op0=mybir.AluOpType.mult,
                            op1=mybir.AluOpType.add,
                        )
                    soff += len(wb) + int(corr_t)
                    if r % 4 == 3:
                        base = (r - 3) % RING
                        dmaeng = nc.gpsimd if hq == "gpsimd" else nc.sync
                        dmaeng.dma_start(
                            hout[:, q * CPC + (r - 3): q * CPC + r + 1, :],
                            ring[:, q, base: base + 4, :],
                        )
    nc.compile()
    _NC_CACHE[key] = nc
    return nc


# ----------------------------------------------------------------------------
# apply program
# ----------------------------------------------------------------------------

def _build_apply_nc(corr_slots, nrhs):
    acc = os.environ.get("DIFFKS_ACC", "vec")   # split | vec
    ncorrmode = os.environ.get("DIFFKS_NCORR", "on")  # on | off
    key = ("A2", tuple(corr_slots), nrhs, acc, ncorrmode)
    if key in _NC_CACHE:
        return _NC_CACHE[key]
    n_corr = max(len(corr_slots), 1)
    nc = bacc.Bacc("TRN2", target_bir_lowering=False, debug=False,
                   num_devices=N_CORES, enable_partition_id=False)
    hseg = nc.dram_tensor("hseg", [CHUNK, CH * CPC, nrhs], F16,
                          kind="ExternalInput")
    wb = nc.dram_tensor("wb", [CHUNK, CH, nrhs], F16, kind="ExternalInput")
    cwts = nc.dram_tensor("cwts", [CORR, n_corr, CHUNK], F16,
                          kind="ExternalInput")
    yout = nc.dram_tensor("yout", [CHUNK, CH * CPC], F32,
                          kind="ExternalOutput")

    with tile.TileContext(nc) as tc:
        with (
            tc.tile_pool(name="state", bufs=1) as state,
            tc.tile_pool(name="hpool", bufs=6) as hpool,
            tc.tile_pool(name="spool", bufs=4) as spool,
            tc.tile_pool(name="lpool", bufs=4) as lpool,
            tc.tile_pool(name="psum", bufs=6, space="PSUM") as ppool,
        ):
            wb_sb = state.tile([CHUNK, CH, nrhs], F16)
            nc.sync.dma_start(wb_sb[:], wb[:])
            cw_sb = state.tile([CORR, n_corr, CHUNK], F16)
            nc.sync.dma_start(cw_sb[:], cwts[:])
            yout_sb = state.tile([CHUNK, CH * CPC], F32)
            # fused multiply + accumulate-reduce per H column, with each
            # corr fix-up issued right after its column (PE/ACT work overlaps
            # the DVE accumulation stream)
            ci_of = {hcol: ci for ci, hcol in enumerate(corr_slots)}
            for grp in range(CH * CPC // 4):
                htile = hpool.tile([CHUNK, 4, nrhs], F16, tag="h")
                nc.sync.dma_start(htile[:],
                                  hseg[:, grp * 4:(grp + 1) * 4, :])
                for j in range(4):
                    hcol = grp * 4 + j
                    q = hcol // CPC
                    scratch = spool.tile([CHUNK, nrhs], F16, tag="s")
                    if acc == "ttr":
                        nc.vector.tensor_tensor_reduce(
                            out=scratch[:], in0=htile[:, j, :],
                            in1=wb_sb[:, q, :], scale=1.0, scalar=0.0,
                            op0=mybir.AluOpType.mult,
                            op1=mybir.AluOpType.add,
                            accum_out=yout_sb[:, hcol: hcol + 1],
                        )
                    else:
                        nc.vector.scalar_tensor_tensor(
                            out=scratch[:], in0=htile[:, j, :], scalar=1.0,
                            in1=wb_sb[:, q, :], op0=mybir.AluOpType.mult,
                            op1=mybir.AluOpType.mult,
                            accum_out=yout_sb[:, hcol: hcol + 1],
                        )
                    if ncorrmode == "on" and hcol in ci_of:
                        ci = ci_of[hcol]
                        ylo = lpool.tile([CORR, 1], F16, tag="lo")
                        nc.scalar.copy(ylo[:], yout_sb[0:CORR, hcol: hcol + 1])
                        psum2 = ppool.tile([CHUNK, 1], F32, tag="c")
                        nc.tensor.matmul(psum2[:], cw_sb[:, ci, :], ylo[:],
                                         start=True, stop=True)
                        # ACT keeps the fix-up off the vector accum stream
                        nc.scalar.activation(
                            out=yout_sb[CORR:, hcol: hcol + 1],
                            in_=psum2[CORR:, :],
                            func=mybir.ActivationFunctionType.Identity,
                            bias=yout_sb[CORR:, hcol: hcol + 1],
                        )
            nc.sync.dma_start(yout[:], yout_sb[:])
    nc.compile()
    _NC_CACHE[key] = nc
    return nc


# ----------------------------------------------------------------------------
# host orchestration
# ----------------------------------------------------------------------------

def _run(nc, in_maps, tag):
    trace = bool(int(os.environ.get("DIFFKS_TRACE", "0")))
    kw = {}
    tcs = os.environ.get("DIFFKS_TRACE_CORES", "")
    if trace and tcs:
        kw["trace_cores"] = [int(x) for x in tcs.split(",")]
    res = run_bass_kernel_spmd(
        nc, in_maps, core_ids=list(range(len(in_maps))), trace=trace, **kw
    )
    LAST_RESULTS[tag] = res
    return res.results


def kernel(delay_len_frames, raw_coeff_frames, excitation, n_samples):
    n = int(n_samples)
    assert n == N_SAMPLES, f"kernel hardcoded for {N_SAMPLES}, got {n}"
    LAST_RESULTS.clear()

    vals, z_l, x = _preprocess(delay_len_frames, raw_coeff_frames,
                               excitation, n)
    wts, basis = _build_wts(vals, z_l, n)
    nr, pidx = _nr_of(basis)
    n_chunks = n // CHUNK
    assert n_chunks == SEGS * CPC
    xin_cols = np.ascontiguousarray(x.reshape(n_chunks, CHUNK).T)  # [128, nc]

    # fold corrections, then negate everything (update becomes plain copy)
    seg_wts_neg = [-_fold_corr(wts[j * CPC:(j + 1) * CPC])
                   for j in range(SEGS)]
    plans, corr_y = _make_plans(seg_wts_neg)
    tot_blocks = sum(len(plans[q][r][0]) + int(plans[q][r][1])
                     for r in range(CPC) for q in range(CH))

    ncB = _build_phaseb_nc(plans, tot_blocks, basis)
    r0 = _basis_ring0(basis, nr).astype(np.float16)
    in_maps = []
    for s in range(N_CORES):
        xin = np.zeros((CHUNK, CH * CPC), np.float32)
        for r in range(CPC):
            for q in range(CH):
                gchunk = _seg_of(s, q) * CPC + r
                xin[:, r * CH + q] = xin_cols[:, gchunk]
        in_maps.append({
            "wts": _pack_weights(seg_wts_neg, plans, s),
            "xin": xin,
            "ring0": r0,
        })
    outsB = _run(ncB, in_maps, "phaseB")

    # host combine: build each segment's transfer operator from its last 4
    # (uncorrected) H columns + the correction blocks, then chain them (fp32)
    wins = [np.zeros(WIN, np.float32)]
    for j in range(SEGS):
        s, q = j % N_CORES, j // N_CORES
        base = q * CPC + (CPC - 4)
        T = outsB[s]["hout"][:, base: base + 4, :].astype(np.float32)
        blocks = seg_wts_neg[j].reshape(CPC, 5, CHUNK, CHUNK)
        for k in range(4):
            Lc = blocks[CPC - 4 + k, 4][0:CORR]      # negated lhsT [64, 128]
            if np.any(Lc):
                fix = Lc.T @ T[0:CORR, k, :]          # [128, nr]
                T[CORR:, k, :] += fix[CORR:]
        T = T.transpose(1, 0, 2).reshape(WIN, nr)
        w_next = T[:, :basis] @ wins[j][WIN - basis:] + T[:, pidx]
        wins.append(w_next.astype(np.float32))

    # apply: y[:, c] = H_c @ [w; 1]
    corr_slots = [q * CPC + r for q in range(CH) for r in range(CPC)
                  if corr_y[q][r]]
    ncA = _build_apply_nc(corr_slots, nr)
    n_corr = max(len(corr_slots), 1)
    in_maps = []
    for s in range(N_CORES):
        wbv = np.zeros((CHUNK, CH, nr), np.float16)
        for q in range(CH):
            j = _seg_of(s, q)
            wv = np.zeros(nr, np.float32)
            wv[:basis] = wins[j][WIN - basis:]
            wv[pidx] = 1.0
            wbv[:, q, :] = wv.astype(np.float16)[None, :]
        cw = np.zeros((CORR, n_corr, CHUNK), np.float16)
        for ci, hcol in enumerate(corr_slots):
            q, r = hcol // CPC, hcol % CPC
            blocks = seg_wts_neg[_seg_of(s, q)].reshape(CPC, 5, CHUNK, CHUNK)
            cw[:, ci, :] = blocks[r, 4][0:CORR].astype(np.float16)
        in_maps.append({
            "hseg": outsB[s]["hout"],
            "wb": wbv,
            "cwts": cw,
        })
    outsA = _run(ncA, in_maps, "apply")

    y = np.zeros(n, np.float32)
    for s in range(N_CORES):
        yo = outsA[s]["yout"]          # [128, CH*CPC]
        for q in range(CH):
            for r in range(CPC):
                gchunk = _seg_of(s, q) * CPC + r
                y[gchunk * CHUNK:(gchunk + 1) * CHUNK] = yo[:, q * CPC + r]
    return y.astype(np.float32)


# revision 23
# speedup vs baseline: 4.9424x; 1.1343x over previous
"""Trainium2 Bass kernel for nn_DiffKS (differentiable Karplus-Strong).

Structure of the computation:
  y[t] = x[t] - sum_{j=0..5} vals[t,j] * y[t - 1 - z_l[t] - j]
with vals / z_l derived from spline-interpolated delay & coefficient
trajectories.  The feedback lag (1 + z_l + j) is always >= ~93 samples, so
128-sample chunks can be computed as dense banded matmuls against a
512-sample window of past output plus a small within-chunk correction.

Parallel structure (v2):
  - 32 time segments of 2048 samples; each of the 8 cores runs 4 segments
    as INDEPENDENT interleaved chunk-chains (4 chains x 16 rounds), which
    keeps the tensor engine busy while each chain's serial
    matmul->ring-update link completes.  All tensors fp16 (weights, ring,
    H) with fp32 PSUM accumulation: host-simulated rel err ~5e-4.
  - phase B (parallel): each chain runs its segment's chunked recurrence
    with basis+1 right-hand sides (unit initial-window columns + one
    particular column).  Ring columns double as the response operator H
    (streamed to DRAM in fp16); the corrected final windows form the
    segment's transfer operator T.
  - combine (host, tiny): chain the 32 transfer operators to get every
    segment's true initial window.
  - apply (parallel): y[:, c] = H_c @ [w; 1] as fused multiply+
    accumulate-reduce ops split across the Vector and GpSimd engines,
    plus within-chunk correction fix-ups.

Weights are pre-negated on the host so the serial ring update is a plain
PSUM->SBUF copy on the Scalar (ACT) engine (plus a [128,1] x-column add on
Vector), keeping the Vector engine off the critical chain.
"""

import os
import numpy as np

import concourse.bacc as bacc
import concourse.tile as tile
import concourse.mybir as mybir
from concourse.bass_utils import run_bass_kernel_spmd


def _ensure_ntff_hook():
    """The agent image's `antenv` stub lacks `axon_hooks`, which
    `run_bass_kernel_spmd(trace=True)` needs under axon for NTFF capture."""
    try:
        from antenv.axon_hooks import get_axon_ntff_profile_hook  # noqa: F401
        return
    except ImportError:
        pass
    import contextlib
    import ctypes
    import sys
    import types

    so_path = "/opt/axon/libaxon_pjrt.so"
    if not os.path.exists(so_path):
        return
    lib = ctypes.CDLL(so_path)
    if not hasattr(lib, "axon_start_nrt_profile"):
        return
    lib.axon_start_nrt_profile.argtypes = [
        ctypes.POINTER(ctypes.c_int64), ctypes.c_size_t]
    lib.axon_start_nrt_profile.restype = ctypes.c_int64
    lib.axon_stop_nrt_profile.argtypes = [ctypes.c_char_p]
    lib.axon_stop_nrt_profile.restype = ctypes.c_int64

    @contextlib.contextmanager
    def _hook(output_dir, device_ids):
        import jax
        jax.devices()
        if device_ids:
            ids = (ctypes.c_int64 * len(device_ids))(*device_ids)
            rc = lib.axon_start_nrt_profile(ids, len(device_ids))
        else:
            rc = lib.axon_start_nrt_profile(None, 0)
        if rc != 0:
            raise RuntimeError(f"axon_start_nrt_profile rc={rc}")
        try:
            yield
        finally:
            n = lib.axon_stop_nrt_profile(str(output_dir).encode())
            if n <= 0:
                print(f"ntff profile: {n} file(s) written to {output_dir}",
                      file=sys.stderr)

    mod = types.ModuleType("antenv.axon_hooks")
    mod._hook = _hook
    mod.get_axon_ntff_profile_hook = lambda: _hook
    mod.set_axon_ntff_profile_hook = lambda h: setattr(mod, "_hook", h)
    import antenv
    antenv.axon_hooks = mod
    sys.modules["antenv.axon_hooks"] = mod


_ensure_ntff_hook()

F32 = mybir.dt.float32
F16 = mybir.dt.float16

N_SAMPLES = 65536
N_FRAMES = 64
L_ORDER = 5
CHUNK = 128
WIN = 512            # window length the chunk matmuls see (4 ring cols)
RING = 8             # ring columns per chain in SBUF
CORR = 64            # within-chunk correction width (needs z_l >= 63)
N_CORES = 8
CH = 4               # independent chains (segments) per core
CPC = 16             # chunks (rounds) per chain
SEGS = N_CORES * CH  # 32 segments of 2048 samples

# filled by kernel() with per-phase profiling results for the test harness
LAST_RESULTS = {}

_NC_CACHE = {}

# device rhs layout: basis columns [0, basis); one zero pad column; the
# particular column at PIDX (4-byte aligned for the fp16 column update);
# one trailing pad so the total width is even.
def _nr_of(basis):
    pidx = basis + 1 + (basis + 1) % 2
    return pidx + 2 - (basis + 1) % 2, pidx


def _seg_of(s, q):
    """Segment index handled by core s, chain q."""
    return s + N_CORES * q


# ----------------------------------------------------------------------------
# host-side preprocessing
# ----------------------------------------------------------------------------

_SPLINE_CACHE = {}


def _spline_matrix(n_in, n_out):
    """Static [n_out, n_in] natural-cubic-spline interpolation matrix."""
    key = (n_in, n_out)
    if key in _SPLINE_CACHE:
        return _SPLINE_CACHE[key]
    t_in = np.linspace(0.0, 1.0, n_in)
    t_out = np.linspace(0.0, 1.0, n_out)
    n = n_in
    h = t_in[1:] - t_in[:-1]
    R = np.zeros((n - 2, n))
    for i in range(n - 2):
        R[i, i] += 6.0 / h[i]
        R[i, i + 1] += -6.0 / h[i] - 6.0 / h[i + 1]
        R[i, i + 2] += 6.0 / h[i + 1]
    A = (
        np.diag(2.0 * (h[:-1] + h[1:]))
        + np.diag(h[1:-1], 1)
        + np.diag(h[1:-1], -1)
    )
    M = np.zeros((n, n))
    M[1:-1] = np.linalg.solve(A, R)
    idx = np.clip(np.searchsorted(t_in, t_out, side="right") - 1, 0, n - 2)
    dt = t_out - t_in[idx]
    S = np.zeros((n_out, n))
    eye = np.eye(n)
    for r in range(n_out):
        i = idx[r]
        b = (eye[i + 1] - eye[i]) / h[i] - h[i] * (2.0 * M[i] + M[i + 1]) / 6.0
        c = M[i] / 2.0
        d = (M[i + 1] - M[i]) / (6.0 * h[i])
        S[r] = eye[i] + b * dt[r] + c * dt[r] ** 2 + d * dt[r] ** 3
    S = S.astype(np.float32)
    _SPLINE_CACHE[key] = S
    return S


def _preprocess(delay, raw, exc, n_samples):
    sig = 1.0 / (1.0 + np.exp(-np.asarray(raw, np.float32)))
    coeff = sig / sig.sum(-1, keepdims=True)
    S = _spline_matrix(N_FRAMES, n_samples)
    delay_interp = S @ np.asarray(delay, np.float32)
    coeff_interp = S @ coeff
    z_l = np.floor(delay_interp).astype(np.int32)
    alfa = (delay_interp - z_l).astype(np.float32)
    b = coeff_interp
    v0 = -(1.0 - alfa) * b[:, 0]
    vmid = -(alfa[:, None] * b[:, : L_ORDER - 1]
             + (1.0 - alfa)[:, None] * b[:, 1:L_ORDER])
    vL = -alfa * b[:, -1]
    vals = np.concatenate([v0[:, None], vmid, vL[:, None]], 1).astype(np.float32)
    x = np.zeros(n_samples, np.float32)
    exc = np.asarray(exc, np.float32)
    x[: exc.shape[0]] = exc
    return vals, z_l, x


def _build_wts(vals, z_l, n_samples):
    """Dense per-chunk matmul weights in lhsT layout.

    wts[c, 128g + p, m] = W[c][m, 128g + p]   (g = 0..3, window blocks)
    wts[c, 512 + p, m]  = L[c][m, p]          (p < 64, correction block)
    """
    n_chunks = n_samples // CHUNK
    t = np.arange(n_samples)
    lag = 1 + z_l[:, None] + np.arange(6)[None, :]
    assert (lag[:, 0] >= CORR).all(), "delay too small for correction width"
    basis = int(lag.max())
    assert basis <= WIN - CORR, "delay too large for window"
    src = t[:, None] - lag
    i_in_chunk = t % CHUNK
    k_win = WIN + i_in_chunk[:, None] - lag
    wts = np.zeros((n_chunks, 5 * CHUNK, CHUNK), np.float32)
    c_of_t = t // CHUNK
    for j in range(6):
        valid = src[:, j] >= 0
        kw = k_win[:, j]
        in_window = valid & (kw < WIN)
        tw = t[in_window]
        wts[c_of_t[tw], kw[tw], i_in_chunk[tw]] += vals[tw, j]
        in_chunk = valid & (kw >= WIN)
        tc = t[in_chunk]
        kc = kw[tc] - WIN
        assert (kc < CORR).all()
        wts[c_of_t[tc], WIN + kc, i_in_chunk[tc]] += vals[tc, j]
    return wts, basis


def _fold_corr(wts_seg):
    """Fold each chunk's within-chunk correction into the weights of its
    in-segment readers so the ring stores *uncorrected* columns."""
    wts_seg = wts_seg.copy()
    n = wts_seg.shape[0]
    blocks = wts_seg.reshape(n, 5, CHUNK, CHUNK)
    corr_active = np.abs(blocks[:, 4]).reshape(n, -1).max(-1) > 0
    for w in range(n):
        if not corr_active[w]:
            continue
        corrT = blocks[w, 4]
        for r in range(w + 1, min(w + 5, n)):
            g = w - r + 4
            blk = blocks[r, g]
            blk[0:CORR] -= corrT[0:CORR, CORR:] @ blk[CORR:]
    return wts_seg


def _basis_ring0(basis, nr):
    """Initial window columns: basis b is a unit at window position
    (WIN-basis)+b; particular and pad columns start at zero."""
    r0 = np.zeros((CHUNK, 4, nr), np.float32)
    for b in range(basis):
        p = (WIN - basis) + b
        r0[p % CHUNK, p // CHUNK, b] = 1.0
    return r0


# ----------------------------------------------------------------------------
# plan construction (shared across cores; SPMD program)
# ----------------------------------------------------------------------------

def _make_plans(seg_wts_neg):
    """Per-slot (q, r) union plans across cores.

    Returns:
      plans[q][r] = (wblocks tuple, corr_t bool)   # phase B
      corr_y[q][r] = bool                          # apply fixup positions
    """
    act = np.stack([
        np.abs(w.reshape(CPC, 5, -1)).max(-1) > 0 for w in seg_wts_neg
    ])  # [SEGS, CPC, 5]
    plans = []
    corr_y = []
    for q in range(CH):
        segs = [_seg_of(s, q) for s in range(N_CORES)]
        u = act[segs].any(0)  # [CPC, 5]
        pq = []
        cq = []
        for r in range(CPC):
            wb = tuple(g for g in range(4) if u[r, g])
            if not wb:
                wb = (3,)
            pq.append((wb, False))
            cq.append(bool(u[r, 4]))
        plans.append(pq)
        corr_y.append(cq)
    return plans, corr_y


def _pack_weights(seg_wts_neg, plans, s):
    """Pack core s's phase-B weight blocks, round-major, partition-major
    fp16 layout [128, TOT, 128]."""
    cols = []
    for r in range(CPC):
        for q in range(CH):
            wb, corr_t = plans[q][r]
            blocks = seg_wts_neg[_seg_of(s, q)].reshape(CPC, 5, CHUNK, CHUNK)
            sel = list(wb) + ([4] if corr_t else [])
            cols.append(blocks[r, sel])  # [nb, 128, 128]
    flat = np.concatenate(cols, 0)       # [TOT, 128k, 128m]
    return np.ascontiguousarray(
        flat.transpose(1, 0, 2)).astype(np.float16)  # [128, TOT, 128]


# ----------------------------------------------------------------------------
# phase B program
# ----------------------------------------------------------------------------

def _plan_key(plans):
    return tuple(tuple((wb, co) for wb, co in pq) for pq in plans)


def _build_phaseb_nc(plans, tot_blocks, basis):
    upd = os.environ.get("DIFFKS_UPD", "act")   # act | stt | vv
    hq = os.environ.get("DIFFKS_HQ", "sync")  # gpsimd | sync
    us_env = os.environ.get("DIFFKS_US1", "216")
    key = ("B2", _plan_key(plans), tot_blocks, basis, upd, hq, us_env)
    if key in _NC_CACHE:
        return _NC_CACHE[key]
    nr, pidx = _nr_of(basis)

    nb_round_max = max(
        sum(len(plans[q][r][0]) + int(plans[q][r][1]) for q in range(CH))
        for r in range(CPC)
    )
    nc = bacc.Bacc("TRN2", target_bir_lowering=False, debug=False,
                   num_devices=N_CORES, enable_partition_id=False)
    wts = nc.dram_tensor("wts", [CHUNK, tot_blocks, CHUNK], F16,
                         kind="ExternalInput")
    xin = nc.dram_tensor("xin", [CHUNK, CH * CPC], F32, kind="ExternalInput")
    ring0 = nc.dram_tensor("ring0", [CHUNK, 4, nr], F16,
                           kind="ExternalInput")
    hout = nc.dram_tensor("hout", [CHUNK, CH * CPC, nr], F16,
                          kind="ExternalOutput")

    with tile.TileContext(nc) as tc:
        with (
            tc.tile_pool(name="state", bufs=1) as state,
            tc.tile_pool(name="wpool", bufs=4) as wpool,
            tc.tile_pool(name="psum", bufs=8, space="PSUM") as ppool,
        ):
            ring = state.tile([CHUNK, CH, RING, nr], F16)
            xin_sb = state.tile([CHUNK, CH * CPC], F32)
            xext = None
            if upd in ("stt", "vv"):
                xext = state.tile([CHUNK, CH, nr], F16)
                nc.vector.memset(xext[:], 0.0)
            woff = 0
            wtile0 = None
            for r in range(CPC):
                nbr = sum(len(plans[q][r][0]) + int(plans[q][r][1])
                          for q in range(CH))
                wtile = wpool.tile([CHUNK, nb_round_max, CHUNK], F16, tag="w")
                if r == 0:
                    # round 0: per-slot weight pieces interleaved with the
                    # ring0 columns so the first matmul starts ~2us earlier
                    so = 0
                    for q in range(CH):
                        nb_q = len(plans[q][0][0]) + int(plans[q][0][1])
                        nc.sync.dma_start(
                            wtile[:, so: so + nb_q, :],
                            wts[:, woff + so: woff + so + nb_q, :])
                        nc.sync.dma_start(ring[:, q, 4:8, :], ring0[:])
                        so += nb_q
                    nc.sync.dma_start(xin_sb[:], xin[:])
                else:
                    nc.sync.dma_start(wtile[:, 0:nbr, :],
                                      wts[:, woff: woff + nbr, :])
                woff += nbr
                soff = 0
                for q in range(CH):
                    wb, corr_t = plans[q][r]
                    slot = r * CH + q
                    rc = r % RING
                    psum = ppool.tile([CHUNK, nr], F32, tag="acc")
                    for i, g in enumerate(wb):
                        col = (r + 4 + g) % RING
                        nc.tensor.matmul(
                            psum[:],
                            wtile[:, soff + i, :],
                            ring[:, q, col, :],
                            start=(i == 0),
                            stop=(i == len(wb) - 1),
                        )
                    # serial ring update (weights pre-negated: col = psum + x).
                    # One balanced copy each on ACT and Vector; x is nonzero
                    # only for the first 4 chunks of segment 0, so only those
                    # slots get an in-place x-add (other cores add zero).
                    if upd == "act":
                        s1 = int(os.environ.get("DIFFKS_US1", "216"))
                        nc.scalar.copy(ring[:, q, rc, 0:s1], psum[:, 0:s1])
                        nc.vector.tensor_copy(ring[:, q, rc, s1:nr],
                                              psum[:, s1:nr])
                        if q == 0 and r < 4:
                            nc.vector.tensor_add(
                                ring[:, q, rc, pidx: pidx + 1],
                                ring[:, q, rc, pidx: pidx + 1],
                                xin_sb[:, slot: slot + 1],
                            )
                    else:
                        if upd == "stt":
                            nc.scalar.copy(xext[:, q, pidx: pidx + 1],
                                           xin_sb[:, slot: slot + 1])
                        else:
                            nc.vector.tensor_copy(xext[:, q, pidx: pidx + 1],
                                                  xin_sb[:, slot: slot + 1])
                        nc.vector.scalar_tensor_tensor(
                            out=ring[:, q, rc, :], in0=psum[:], scalar=1.0,
                            in1=xext[:, q, :], op0=mybir.AluOpType.mult,
                            op1=mybir.AluOpType.add,
                        )
                    soff += len(wb) + int(corr_t)
                    if r % 4 == 3:
                        base = (r - 3) % RING
                        dmaeng = nc.gpsimd if hq == "gpsimd" else nc.sync
                        dmaeng.dma_start(
                            hout[:, q * CPC + (r - 3): q * CPC + r + 1, :],
                            ring[:, q, base: base + 4, :],
                        )
    nc.compile()
    _NC_CACHE[key] = nc
    return nc


# ----------------------------------------------------------------------------
# apply program
# ----------------------------------------------------------------------------

def _build_apply_nc(corr_slots, nrhs):
    acc = os.environ.get("DIFFKS_ACC", "vec")   # tri | vec
    ncorrmode = os.environ.get("DIFFKS_NCORR", "on")  # on | off
    key = ("A2", tuple(corr_slots), nrhs, acc, ncorrmode)
    if key in _NC_CACHE:
        return _NC_CACHE[key]
    n_corr = max(len(corr_slots), 1)
    nc = bacc.Bacc("TRN2", target_bir_lowering=False, debug=False,
                   num_devices=N_CORES, enable_partition_id=False)
    hseg = nc.dram_tensor("hseg", [CHUNK, CH * CPC, nrhs], F16,
                          kind="ExternalInput")
    wb = nc.dram_tensor("wb", [CHUNK, CH, nrhs], F16, kind="ExternalInput")
    yout = nc.dram_tensor("yout", [CHUNK, CH * CPC], F32,
                          kind="ExternalOutput")

    with tile.TileContext(nc) as tc:
        with (
            tc.tile_pool(name="state", bufs=1) as state,
            tc.tile_pool(name="hpool", bufs=6) as hpool,
            tc.tile_pool(name="spool", bufs=4) as spool,
        ):
            wb_sb = state.tile([CHUNK, CH, nrhs], F16)
            nc.sync.dma_start(wb_sb[:], wb[:])
            yout_sb = state.tile([CHUNK, CH * CPC], F32)
            # fused multiply + accumulate-reduce per H column (corr fix-ups
            # are applied on the host after yout returns)
            for grp in range(CH * CPC // 4):
                htile = hpool.tile([CHUNK, 4, nrhs], F16, tag="h")
                nc.sync.dma_start(htile[:],
                                  hseg[:, grp * 4:(grp + 1) * 4, :])
                for j in range(4):
                    hcol = grp * 4 + j
                    q = hcol // CPC
                    scratch = spool.tile([CHUNK, nrhs], F16, tag="s")
                    if acc == "tri" and hcol % 3 == 0:
                        # gpsimd computes the products, ACT reduce-accumulates
                        nc.gpsimd.scalar_tensor_tensor(
                            out=scratch[:], in0=htile[:, j, :], scalar=1.0,
                            in1=wb_sb[:, q, :], op0=mybir.AluOpType.mult,
                            op1=mybir.AluOpType.mult,
                        )
                        scr2 = spool.tile([CHUNK, nrhs], F16, tag="s2")
                        nc.scalar.activation(
                            out=scr2[:], in_=scratch[:],
                            func=mybir.ActivationFunctionType.Identity,
                            accum_out=yout_sb[:, hcol: hcol + 1],
                        )
                    else:
                        nc.vector.scalar_tensor_tensor(
                            out=scratch[:], in0=htile[:, j, :], scalar=1.0,
                            in1=wb_sb[:, q, :], op0=mybir.AluOpType.mult,
                            op1=mybir.AluOpType.mult,
                            accum_out=yout_sb[:, hcol: hcol + 1],
                        )
            nc.sync.dma_start(yout[:], yout_sb[:])
    nc.compile()
    _NC_CACHE[key] = nc
    return nc


# ----------------------------------------------------------------------------
# host orchestration
# ----------------------------------------------------------------------------

def _run(nc, in_maps, tag):
    trace = bool(int(os.environ.get("DIFFKS_TRACE", "0")))
    kw = {}
    tcs = os.environ.get("DIFFKS_TRACE_CORES", "")
    if trace and tcs:
        kw["trace_cores"] = [int(x) for x in tcs.split(",")]
    res = run_bass_kernel_spmd(
        nc, in_maps, core_ids=list(range(len(in_maps))), trace=trace, **kw
    )
    LAST_RESULTS[tag] = res
    return res.results


def kernel(delay_len_frames, raw_coeff_frames, excitation, n_samples):
    n = int(n_samples)
    assert n == N_SAMPLES, f"kernel hardcoded for {N_SAMPLES}, got {n}"
    LAST_RESULTS.clear()

    vals, z_l, x = _preprocess(delay_len_frames, raw_coeff_frames,
                               excitation, n)
    wts, basis = _build_wts(vals, z_l, n)
    nr, pidx = _nr_of(basis)
    n_chunks = n // CHUNK
    assert n_chunks == SEGS * CPC
    xin_cols = np.ascontiguousarray(x.reshape(n_chunks, CHUNK).T)  # [128, nc]

    # fold corrections, then negate everything (update becomes plain copy)
    seg_wts_neg = [-_fold_corr(wts[j * CPC:(j + 1) * CPC])
                   for j in range(SEGS)]
    plans, corr_y = _make_plans(seg_wts_neg)
    tot_blocks = sum(len(plans[q][r][0]) + int(plans[q][r][1])
                     for r in range(CPC) for q in range(CH))

    ncB = _build_phaseb_nc(plans, tot_blocks, basis)
    r0 = _basis_ring0(basis, nr).astype(np.float16)
    in_maps = []
    for s in range(N_CORES):
        xin = np.zeros((CHUNK, CH * CPC), np.float32)
        for r in range(CPC):
            for q in range(CH):
                gchunk = _seg_of(s, q) * CPC + r
                xin[:, r * CH + q] = xin_cols[:, gchunk]
        in_maps.append({
            "wts": _pack_weights(seg_wts_neg, plans, s),
            "xin": xin,
            "ring0": r0,
        })
    outsB = _run(ncB, in_maps, "phaseB")

    # host combine: build each segment's transfer operator from its last 4
    # (uncorrected) H columns + the correction blocks, then chain them (fp32)
    wins = [np.zeros(WIN, np.float32)]
    for j in range(SEGS):
        s, q = j % N_CORES, j // N_CORES
        base = q * CPC + (CPC - 4)
        T = outsB[s]["hout"][:, base: base + 4, :].astype(np.float32)
        blocks = seg_wts_neg[j].reshape(CPC, 5, CHUNK, CHUNK)
        for k in range(4):
            Lc = blocks[CPC - 4 + k, 4][0:CORR]      # negated lhsT [64, 128]
            if np.any(Lc):
                fix = Lc.T @ T[0:CORR, k, :]          # [128, nr]
                T[CORR:, k, :] += fix[CORR:]
        T = T.transpose(1, 0, 2).reshape(WIN, nr)
        w_next = T[:, :basis] @ wins[j][WIN - basis:] + T[:, pidx]
        wins.append(w_next.astype(np.float32))

    # apply: y[:, c] = H_c @ [w; 1]
    corr_slots = [q * CPC + r for q in range(CH) for r in range(CPC)
                  if corr_y[q][r]]
    ncA = _build_apply_nc(corr_slots, nr)
    n_corr = max(len(corr_slots), 1)
    in_maps = []
    for s in range(N_CORES):
        wbv = np.zeros((CHUNK, CH, nr), np.float16)
        for q in range(CH):
            j = _seg_of(s, q)
            wv = np.zeros(nr, np.float32)
            wv[:basis] = wins[j][WIN - basis:]
            wv[pidx] = 1.0
            wbv[:, q, :] = wv.astype(np.float16)[None, :]
        in_maps.append({
            "hseg": outsB[s]["hout"],
            "wb": wbv,
        })
    outsA = _run(ncA, in_maps, "apply")

    y = np.zeros(n, np.float32)
    for s in range(N_CORES):
        yo = np.array(outsA[s]["yout"])          # [128, CH*CPC]
        for q in range(CH):
            blocks = seg_wts_neg[_seg_of(s, q)].reshape(CPC, 5, CHUNK, CHUNK)
            for r in range(CPC):
                hcol = q * CPC + r
                Lc = blocks[r, 4][0:CORR]        # negated lhsT [64, 128]
                if np.any(Lc):
                    fix = Lc.T @ yo[0:CORR, hcol]
                    yo[CORR:, hcol] += fix[CORR:]
                gchunk = _seg_of(s, q) * CPC + r
                y[gchunk * CHUNK:(gchunk + 1) * CHUNK] = yo[:, hcol]
    return y.astype(np.float32)
